# revision 1
# baseline (speedup 1.0000x reference)
"""GCN2 (nn_GCN2_42331197669873) Bass kernel for 8 TRN2 NeuronCores.

Strategy: graph/data parallel. Nodes sharded row-wise across 8 cores
(12500 each). Per layer:
  - AllGather the node features into a full replica in each core's HBM.
  - Sparse propagate: edges bucketed by destination core; within a core,
    edges are assigned to 14 fixed source-windows (7143 rows each) so the
    SWDGE dma_gather can address rows with int16 indices relative to a
    compile-time window base shared by all cores (SPMD).  Gathered rows are
    multiplied by edge weights (per-partition scalars on the ACT engine)
    and scatter-added into per-core HBM aggregation buffers with
    dma_scatter_add.  Duplicate dst indices inside one scatter call lose
    adds on HW (read-modify-writes pipeline), so tokens in each batch are
    grouped into occurrence-runs: run 1 (first occurrence of each dst) goes
    to buffer A, runs >=2 go to buffer B; calls targeting the same buffer
    serialize via Tile's WAW tracking, different buffers run in parallel.
  - Class-center branch: centers = sum_cores (D_shard^T @ (x + 0.1 x0))
    with D = onehot(label)/cnt precomputed on host, tiny [47,128]
    AllReduce, then a Gram-matrix formulation that never materializes the
    [47,47,128] pair tensor: rcm_i = rowsum(a)_i * c_i - (a @ c)_i with
    a = cm / pairdist.
  - Dense phase in feature-major layout (h on partitions) so conv / p@r
    matmuls stream node tiles against stationary weights.

kernel(**inputs) takes the FULL unsharded inputs and returns the FULL
[100000, 47] output; sharding + all preprocessing happens on host inside.
"""

import math
import numpy as np

from concourse import bass, bacc, tile, mybir, bass_utils
from concourse import library_config
from concourse.mybir import AxisListType
import concourse.tile_sem_assignment as _tsa
from concourse import bass_isa as _bisa

# Tile round-robins Pool-engine DMAs over all DMASW sem lanes ignoring
# queue_num; mixing SWDGE queues on one lane breaks its in-order-completion
# assumption (sim: "sem locked to SWDGE queue"). Segregate lanes by queue:
# queue 0 -> lanes [0,4), queue 1 -> lanes [4,8).
_orig_assign_tick = _tsa.TileClockTick._assign_tick

def _assign_tick_qsplit(self, inst):
    if (isinstance(inst, _tsa.DMAInst)
            and inst.engine == mybir.EngineType.Pool
            and not isinstance(inst, _bisa.UserSyncedRemoteDMADescs)
            and self.swdge_sem_count >= 2):
        qn = getattr(inst, "queue_num", 0) or 0
        half = self.swdge_sem_count // 2
        if not hasattr(self, "_qrr"):
            self._qrr = {}
        r = self._qrr.get(qn, 0)
        self._qrr[qn] = r + 1
        self.next_sw_dma_idx = (qn % 2) * half + r % half
    return _orig_assign_tick(self, inst)

_tsa.TileClockTick._assign_tick = _assign_tick_qsplit

F32 = mybir.dt.float32
BF16 = mybir.dt.bfloat16
I16 = mybir.dt.int16


class Cfg:
    def __init__(self, N=100000, E=800000, C=47, H=128, ncores=8, nbatch=14,
                 nb_tok=8192, L=4, alpha=0.1, theta=0.5, rsl=0.5):
        self.N, self.E, self.C, self.H = N, E, C, H
        self.ncores = ncores
        self.NS = N // ncores                 # nodes per core
        self.NT = (self.NS + 127) // 128      # node tiles per core
        self.nbatch = nbatch                  # src windows
        self.W = (N + nbatch - 1) // nbatch   # window width (int16-safe rel idx)
        assert self.W + 256 < 32768
        self.NB = nb_tok                      # tokens per batch (128-mult)
        self.L, self.alpha, self.theta, self.rsl = L, alpha, theta, rsl
        self.trash = self.NT * 128            # scatter trash row (pads)
        self.nagg = self.NT * 128 + 128       # agg rows incl trash row


DEF = Cfg()


# ----------------------------------------------------------------------
# host-side edge preprocessing
# ----------------------------------------------------------------------

def _prep_edges(cfg, edge_index, edge_weight):
    """Per core: batch/window/occurrence-run token layout.

    Returns gidx [nc, nbatch, 128, NB//16] i16, sidx (same), wgt
    [nc, nbatch, 128, NB//128] f32, runs: list of (c0, c1, buf) column
    ranges shared by all cores.
    """
    src = np.asarray(edge_index[0], np.int64)
    dst = np.asarray(edge_index[1], np.int64)
    w = np.asarray(edge_weight, np.float32)
    nc, NS, NB, nbatch, W = cfg.ncores, cfg.NS, cfg.NB, cfg.nbatch, cfg.W

    # token lists per (core, batch, occ-class)
    per_cb = [[None] * nbatch for _ in range(nc)]
    kmax_all = 1
    for c in range(nc):
        m = (dst >= c * NS) & (dst < (c + 1) * NS)
        s_c, d_c, w_c = src[m], dst[m] - c * NS, w[m]
        b_c = s_c // W
        for b in range(nbatch):
            mb = b_c == b
            s_b, d_b, w_b = s_c[mb], d_c[mb], w_c[mb]
            # occurrence index per dst within the batch
            order = np.argsort(d_b, kind="stable")
            s_b, d_b, w_b = s_b[order], d_b[order], w_b[order]
            occ = np.zeros(len(d_b), np.int64)
            if len(d_b):
                is_new = np.ones(len(d_b), bool)
                is_new[1:] = d_b[1:] != d_b[:-1]
                # running count within equal-dst group
                grp_start = np.maximum.accumulate(np.where(is_new, np.arange(len(d_b)), 0))
                occ = np.arange(len(d_b)) - grp_start
                kmax_all = max(kmax_all, int(occ.max()) + 1)
            per_cb[c][b] = (s_b, d_b, w_b, occ)

    kmax = kmax_all
    # static run sizes: max over cores of each (batch, occ) count, 128-aligned
    run_cols = np.zeros((nbatch, kmax), np.int64)
    for b in range(nbatch):
        for k in range(kmax):
            mx = max(int((per_cb[c][b][3] == k).sum()) for c in range(nc))
            run_cols[b, k] = (mx + 127) // 128
    assert run_cols.sum(1).max() * 128 <= NB, (
        f"batch overflow: {run_cols.sum(1).max() * 128} > {NB}")

    # per-batch scatter sub-calls (c0, c1, bufid), each <= CAP_COLS columns
    # (SWDGE calls above ~1024 tokens crash the Q7/device).
    CAP_COLS = 8
    runs = []  # per batch: list of (c0, c1, bufid)
    for b in range(nbatch):
        rb, c0 = [], 0
        for k in range(kmax):
            n = int(run_cols[b, k])
            if n == 0:
                continue
            bufid = 0 if k == 0 else 1
            for s0 in range(0, n, CAP_COLS):
                s1 = min(s0 + CAP_COLS, n)
                rb.append((c0 + s0, c0 + s1, bufid))
            c0 += n
        runs.append(rb)

    gidx = np.zeros((nc, nbatch, 128, NB // 16), np.int16)
    sidx = np.full((nc, nbatch, 128, NB // 16), cfg.trash, np.int16)
    wgt = np.zeros((nc, nbatch, 128, NB // 128), np.float32)
    ti = np.arange(NB)
    rows = (ti % 16)[None, :] + 16 * np.arange(8)[:, None]  # [8, NB]
    cols = ti // 16
    wrow, wcol = ti % 128, ti // 128
    for c in range(nc):
        for b in range(nbatch):
            s_b, d_b, w_b, occ = per_cb[c][b]
            g_lin = np.zeros(NB, np.int16)          # pads gather row 0 of window
            s_lin = np.full(NB, cfg.trash, np.int16)
            w_lin = np.zeros(NB, np.float32)
            c0 = 0
            for k in range(kmax):
                n = int(run_cols[b, k])
                if n == 0:
                    continue
                mk = occ == k
                cnt = int(mk.sum())
                t0 = c0 * 128
                g_lin[t0:t0 + cnt] = (s_b[mk] - b * W).astype(np.int16)
                s_lin[t0:t0 + cnt] = d_b[mk].astype(np.int16)
                w_lin[t0:t0 + cnt] = w_b[mk]
                c0 += n
            for g in range(8):
                gidx[c, b, rows[g], cols] = g_lin
                sidx[c, b, rows[g], cols] = s_lin
            wgt[c, b, wrow, wcol] = w_lin
    return gidx, sidx, wgt, runs


# ----------------------------------------------------------------------
# device program
# ----------------------------------------------------------------------

def build_nc(cfg):
    c = cfg
    nc = bacc.Bacc(None, target_bir_lowering=False, debug=False,
                   num_swdge_queues=2)
    NT, NB, NS, C_, H = c.NT, c.NB, c.NS, c.C, c.H
    nbw = NB // 128          # w / gtile blocks per batch
    L = c.L

    def dram_in(name, shape, dt=F32):
        return nc.declare_dram_parameter(name, shape, dt, isOutput=False)

    xin_t = dram_in("xin_t", [H, NS])
    d_t = dram_in("d_t", [NT, 128, C_])
    p_t = dram_in("p_t", [NT, C_, 128])
    gidx = dram_in("gidx", [c.nbatch, 128, NB // 16], I16)
    sidx = dram_in("sidx", [c.nbatch, 128, NB // 16], I16)
    wgt = dram_in("wgt", [c.nbatch, 128, nbw])
    lin0w = dram_in("lin0w", [H, H])
    lin0b = dram_in("lin0b", [H, 1])
    lin1w = dram_in("lin1w", [H, C_])
    lin1b = dram_in("lin1b", [C_, 1])
    convw = dram_in("convw", [L, H, H])
    cma = dram_in("cma", [C_, C_])
    cmat = dram_in("cmat", [C_, C_])
    i47 = dram_in("i47", [C_, C_])
    ident = dram_in("ident", [128, 128])
    out_t = nc.declare_dram_parameter("out_t", [C_, NS], F32, isOutput=True)

    # internal DRAM
    x_rep = nc.dram_tensor("x_rep", [c.N, H], BF16, addr_space="Shared")
    x_sh = [nc.dram_tensor(f"x_sh{i}", [NS, H], BF16) for i in range(2)]
    x0_sh = nc.dram_tensor("x0_sh", [NS, H], BF16)
    x_T = [nc.dram_tensor(f"x_T{i}", [NT, 128, 128], F32) for i in range(2)]
    x0_T = nc.dram_tensor("x0_T", [NT, 128, 128], F32)
    agg = [nc.dram_tensor(f"agg{i}", [c.nagg, H], BF16) for i in range(2)]
    cen_in = nc.dram_tensor("cen_in", [C_, H], F32)
    cen_out = nc.dram_tensor("cen_out", [C_, H], F32, addr_space="Shared")

    rg = [list(range(c.ncores))]
    betas = [float(np.log(c.theta / (i + 1) + 1.0)) for i in range(L)]

    def tsize(t):
        return min(128, NS - t * 128)

    with tile.TileContext(nc) as tc:
        nc.gpsimd.load_library(library_config.mlp)
        with (
            tc.tile_pool(name="const", bufs=1) as cpool,
            tc.tile_pool(name="sb", bufs=3) as pool,
            tc.tile_pool(name="gt", bufs=2) as gpool,
            tc.tile_pool(name="ps", bufs=3, space="PSUM") as psum,
            tc.tile_pool(name="psacc", bufs=1, space="PSUM") as psacc,
        ):
            # ---- resident constants ----
            lin0w_sb = cpool.tile([H, H], F32)
            nc.sync.dma_start(lin0w_sb[:], lin0w[:, :])
            lin0b_sb = cpool.tile([H, 1], F32)
            nc.sync.dma_start(lin0b_sb[:], lin0b[:, :])
            lin1w_sb = cpool.tile([H, C_], F32)
            nc.sync.dma_start(lin1w_sb[:], lin1w[:, :])
            lin1b_sb = cpool.tile([C_, 1], F32)
            nc.sync.dma_start(lin1b_sb[:], lin1b[:, :])
            convw_sb = cpool.tile([H, L * H], F32)
            for i in range(L):
                nc.sync.dma_start(convw_sb[:, i * H:(i + 1) * H], convw[i])
            cma_sb = cpool.tile([C_, C_], F32)
            nc.sync.dma_start(cma_sb[:], cma[:, :])
            cmat_sb = cpool.tile([C_, C_], F32)
            nc.sync.dma_start(cmat_sb[:], cmat[:, :])
            i47_sb = cpool.tile([C_, C_], F32)
            nc.sync.dma_start(i47_sb[:], i47[:, :])
            ident_sb = cpool.tile([128, 128], F32)
            nc.sync.dma_start(ident_sb[:], ident[:, :])
            zero_sb = cpool.tile([128, 1664], BF16)
            nc.vector.memset(zero_sb[:], 0.0)
            identb_sb = cpool.tile([128, 128], BF16)
            nc.vector.tensor_copy(identb_sb[:], ident_sb[:])

            # ---- lin0: x0 = relu(x @ W0 + b0), write x0_T + x0_sh ----
            for t in range(NT):
                P = tsize(t)
                xi = pool.tile([H, 128], F32, tag="xi")
                nc.sync.dma_start(xi[:, :P], xin_t[:, t * 128:t * 128 + P])
                ps0 = psum.tile([H, 128], F32, tag="b")
                nc.tensor.matmul(ps0[:, :P], lin0w_sb[:], xi[:, :P],
                                 start=True, stop=True)
                x0t = pool.tile([H, 128], F32, tag="x0t")
                nc.scalar.activation(x0t[:, :P], ps0[:, :P],
                                     mybir.ActivationFunctionType.Relu,
                                     bias=lin0b_sb[:, 0:1])
                nc.sync.dma_start(x0_T[t][:, :P], x0t[:, :P])
                x0b = pool.tile([H, 128], BF16, tag="x0b")
                nc.vector.tensor_copy(x0b[:, :P], x0t[:, :P])
                pst = psum.tile([128, 128], BF16, tag="bb")
                nc.tensor.transpose(pst[:P, :], x0b[:, :P], identb_sb[:])
                x0n = pool.tile([128, H], BF16, tag="x0n")
                nc.vector.tensor_copy(x0n[:P, :], pst[:P, :])
                nc.sync.dma_start(x0_sh[t * 128:t * 128 + P, :], x0n[:P, :])

            # ---- layers ----
            for li in range(L):
                beta = betas[li]
                cur_sh = x0_sh if li == 0 else x_sh[li % 2]
                cur_T = x0_T if li == 0 else x_T[li % 2]
                nxt_sh = x_sh[(li + 1) % 2]
                nxt_T = x_T[(li + 1) % 2]

                # - allgather x -
                nc.gpsimd.collective_compute(
                    "AllGather", mybir.AluOpType.bypass, replica_groups=rg,
                    ins=[cur_sh.ap().opt()], outs=[x_rep.ap().opt()],
                )

                # - zero agg buffers -
                for a in range(2):
                    step = 1664
                    for r0 in range(0, c.nagg, step):
                        r1 = min(r0 + step, c.nagg)
                        nc.sync.dma_start(agg[a][r0:r1, :],
                                          zero_sb[:, :r1 - r0])

                # - sparse propagate -
                for b in range(c.nbatch):
                    base = b * c.W
                    wrows = min(c.W + 256, c.N - base)
                    gi = pool.tile([128, NB // 16], I16, tag="gi")
                    nc.sync.dma_start(gi[:], gidx[b])
                    si = pool.tile([128, NB // 16], I16, tag="si")
                    nc.sync.dma_start(si[:], sidx[b])
                    wt = pool.tile([128, nbw], F32, tag="wt")
                    nc.sync.dma_start(wt[:], wgt[b])
                    gtile = gpool.tile([128, nbw, H], BF16, tag="g")
                    # Only the columns actually referenced by scatter runs
                    # need gathering; trailing batch columns are pure pads.
                    used = max(c1 for (_, c1, _) in RUNS[b])
                    # SWDGE calls cap at 1024 tokens (8 cols) - larger
                    # calls wedge the device.
                    for g0 in range(0, used, 8):
                        g1 = min(g0 + 8, nbw)
                        ntok = (g1 - g0) * 128
                        nc.gpsimd.dma_gather(
                            gtile[:, g0:g1, :], x_rep[base:base + wrows, :],
                            gi[:, g0 * 8:g1 * 8],
                            num_idxs=ntok, num_idxs_reg=ntok, elem_size=H,
                            queue_num=1,
                        )
                    for j in range(used):
                        nc.scalar.activation(
                            gtile[:, j, :], gtile[:, j, :],
                            mybir.ActivationFunctionType.Copy,
                            scale=wt[:, j:j + 1],
                        )
                    for (c0, c1, bufid) in RUNS[b]:
                        ntok = (c1 - c0) * 128
                        nc.gpsimd.dma_scatter_add(
                            agg[bufid][:, :], gtile[:, c0:c1, :],
                            si[:, c0 * 8:c1 * 8],
                            num_idxs=ntok, num_idxs_reg=ntok, elem_size=H,
                            queue_num=1,
                        )

                # - pass A: centers partial -
                psA = psacc.tile([C_, H], F32, tag="cen")
                for t in range(NT):
                    P = tsize(t)
                    xa = pool.tile([128, H], BF16, tag="xa")
                    nc.sync.dma_start(xa[:P, :], cur_sh[t * 128:t * 128 + P, :])
                    x0a = pool.tile([128, H], BF16, tag="x0a")
                    nc.sync.dma_start(x0a[:P, :], x0_sh[t * 128:t * 128 + P, :])
                    s = pool.tile([128, H], F32, tag="s")
                    nc.vector.tensor_scalar(s[:P, :], x0a[:P, :], 0.1, None,
                                            mybir.AluOpType.mult)
                    nc.vector.tensor_add(s[:P, :], s[:P, :], xa[:P, :])
                    dt_ = pool.tile([128, C_], F32, tag="dt")
                    nc.sync.dma_start(dt_[:P, :], d_t[t][:P, :])
                    nc.tensor.matmul(psA[:], dt_[:P, :], s[:P, :],
                                     start=(t == 0), stop=(t == NT - 1))
                cenp = pool.tile([C_, H], F32, tag="cenp")
                nc.vector.tensor_copy(cenp[:], psA[:])
                nc.sync.dma_start(cen_in[:, :], cenp[:])
                nc.gpsimd.collective_compute(
                    "AllReduce", mybir.AluOpType.add, replica_groups=rg,
                    ins=[cen_in.ap().opt()], outs=[cen_out.ap().opt()],
                )
                cen = pool.tile([C_, H], F32, tag="cen_sb")
                nc.sync.dma_start(cen[:], cen_out[:, :])

                # - r_cls from centers (Gram trick) -
                pst = psum.tile([128, C_], F32, tag="b")
                nc.tensor.transpose(pst[:, :], cen[:], ident_sb[:C_, :C_])
                cT = pool.tile([128, C_], F32, tag="cT")
                nc.vector.tensor_copy(cT[:], pst[:, :])
                psg = psum.tile([C_, C_], F32, tag="b")
                nc.tensor.matmul(psg[:], cT[:], cT[:], start=True, stop=True)
                g = pool.tile([C_, C_], F32, tag="gg")
                nc.vector.tensor_copy(g[:], psg[:])
                gd = pool.tile([C_, C_], F32, tag="gd")
                nc.vector.tensor_mul(gd[:], g[:], i47_sb[:])
                n2 = pool.tile([C_, 1], F32, tag="n2")
                nc.vector.reduce_sum(n2[:], gd[:], AxisListType.X)
                t1 = pool.tile([C_, C_], F32, tag="t1")
                nc.vector.tensor_scalar(t1[:], g[:], -1.0, n2[:, 0:1],
                                        mybir.AluOpType.mult,
                                        mybir.AluOpType.add)
                ps1 = psum.tile([C_, C_], F32, tag="b")
                nc.tensor.transpose(ps1[:], t1[:], ident_sb[:C_, :C_])
                nrm = pool.tile([C_, C_], F32, tag="nrm")
                nc.vector.tensor_add(nrm[:], t1[:], ps1[:])
                nc.vector.tensor_relu(nrm[:], nrm[:])
                nc.vector.tensor_add(nrm[:], nrm[:], i47_sb[:])
                rn = pool.tile([C_, C_], F32, tag="rn")
                nc.scalar.sqrt(rn[:], nrm[:])
                inv = pool.tile([C_, C_], F32, tag="inv")
                nc.vector.reciprocal(inv[:], rn[:])
                amat = pool.tile([C_, C_], F32, tag="amat")
                nc.vector.tensor_mul(amat[:], cma_sb[:], inv[:])
                atm = pool.tile([C_, C_], F32, tag="atm")
                nc.vector.tensor_mul(atm[:], cmat_sb[:], inv[:])
                rs = pool.tile([C_, 1], F32, tag="rs")
                nc.vector.reduce_sum(rs[:], amat[:], AxisListType.X)
                psm = psum.tile([C_, H], F32, tag="b")
                nc.tensor.matmul(psm[:], atm[:], cen[:], start=True, stop=True)
                rcls = pool.tile([C_, H], F32, tag="rcls")
                nc.vector.tensor_scalar(rcls[:], cen[:], rs[:, 0:1], None,
                                        mybir.AluOpType.mult)
                nc.vector.tensor_sub(rcls[:], rcls[:], psm[:])

                # - pass B -
                for t in range(NT):
                    P = tsize(t)
                    pt = pool.tile([C_, 128], F32, tag="pt")
                    nc.sync.dma_start(pt[:], p_t[t])
                    ps1b = psum.tile([H, 128], F32, tag="b")
                    nc.tensor.matmul(ps1b[:, :P], rcls[:], pt[:, :P],
                                     start=True, stop=True)
                    aA = pool.tile([128, H], BF16, tag="aA")
                    nc.sync.dma_start(aA[:P, :], agg[0][t * 128:t * 128 + P, :])
                    aB = pool.tile([128, H], BF16, tag="aB")
                    nc.sync.dma_start(aB[:P, :], agg[1][t * 128:t * 128 + P, :])
                    aS = pool.tile([128, H], F32, tag="aS")
                    nc.vector.tensor_add(aS[:P, :], aA[:P, :], aB[:P, :])
                    psT = psum.tile([H, 128], F32, tag="b")
                    nc.tensor.transpose(psT[:, :P], aS[:P, :],
                                        ident_sb[:P, :P])
                    xt = pool.tile([H, 128], F32, tag="xt")
                    nc.sync.dma_start(xt[:, :P], cur_T[t][:, :P])
                    x0t2 = pool.tile([H, 128], F32, tag="x0t2")
                    nc.sync.dma_start(x0t2[:, :P], x0_T[t][:, :P])
                    u = pool.tile([H, 128], F32, tag="u")
                    # u = 0.45*(x + aggT + r) + 0.1*x0
                    nc.vector.tensor_add(u[:, :P], xt[:, :P], psT[:, :P])
                    nc.vector.tensor_add(u[:, :P], u[:, :P], ps1b[:, :P])
                    nc.vector.tensor_scalar(u[:, :P], u[:, :P], 0.45, None,
                                            mybir.AluOpType.mult)
                    ux = pool.tile([H, 128], F32, tag="ux")
                    nc.vector.tensor_scalar(ux[:, :P], x0t2[:, :P], 0.1, None,
                                            mybir.AluOpType.mult)
                    nc.vector.tensor_add(u[:, :P], u[:, :P], ux[:, :P])
                    ps2 = psum.tile([H, 128], F32, tag="b")
                    nc.tensor.matmul(ps2[:, :P],
                                     convw_sb[:, li * H:(li + 1) * H],
                                     u[:, :P], start=True, stop=True)
                    o1 = pool.tile([H, 128], F32, tag="o1")
                    nc.vector.tensor_scalar(o1[:, :P], ps2[:, :P], beta, None,
                                            mybir.AluOpType.mult)
                    nc.vector.tensor_scalar(u[:, :P], u[:, :P], 1.0 - beta,
                                            None, mybir.AluOpType.mult)
                    nc.vector.tensor_add(u[:, :P], u[:, :P], o1[:, :P])
                    xn = pool.tile([H, 128], F32, tag="xn")
                    nc.scalar.activation(xn[:, :P], u[:, :P],
                                         mybir.ActivationFunctionType.Relu)
                    nc.sync.dma_start(nxt_T[t][:, :P], xn[:, :P])
                    xnb = pool.tile([H, 128], BF16, tag="xnb")
                    nc.vector.tensor_copy(xnb[:, :P], xn[:, :P])
                    psn = psum.tile([128, 128], BF16, tag="bb")
                    nc.tensor.transpose(psn[:P, :], xnb[:, :P], identb_sb[:])
                    xnn = pool.tile([128, H], BF16, tag="xnn")
                    nc.vector.tensor_copy(xnn[:P, :], psn[:P, :])
                    nc.sync.dma_start(nxt_sh[t * 128:t * 128 + P, :], xnn[:P, :])

            # ---- lin1 ----
            fin_T = x_T[L % 2]
            for t in range(NT):
                P = tsize(t)
                xt = pool.tile([H, 128], F32, tag="fxt")
                nc.sync.dma_start(xt[:, :P], fin_T[t][:, :P])
                psf = psum.tile([C_, 128], F32, tag="b")
                nc.tensor.matmul(psf[:, :P], lin1w_sb[:], xt[:, :P],
                                 start=True, stop=True)
                ot = pool.tile([C_, 128], F32, tag="ot")
                nc.vector.tensor_scalar(ot[:, :P], psf[:, :P],
                                        lin1b_sb[:, 0:1], None,
                                        mybir.AluOpType.add)
                nc.sync.dma_start(out_t[:, t * 128:t * 128 + P], ot[:, :P])

    nc.compile()
    return nc


# RUNS is read by build_nc (static run layout shared across cores)
RUNS = None


# ----------------------------------------------------------------------
# host wrapper
# ----------------------------------------------------------------------

def _prep_inputs(cfg, inputs):
    c = cfg
    x = np.asarray(inputs["x"], np.float32)
    label = np.asarray(inputs["label"], np.int64)
    p = np.asarray(inputs["p"], np.float32)
    cm = np.asarray(inputs["cm"], np.float32)
    lin0_w = np.asarray(inputs["lin0_w"], np.float32)
    lin0_b = np.asarray(inputs["lin0_b"], np.float32)
    lin1_w = np.asarray(inputs["lin1_w"], np.float32)
    lin1_b = np.asarray(inputs["lin1_b"], np.float32)
    conv_w = np.asarray(inputs["conv_w"], np.float32)

    gidx, sidx, wgt, runs = _prep_edges(cfg, inputs["edge_index"],
                                        inputs["edge_weight"])

    cnt = np.bincount(label, minlength=c.C).astype(np.float32)
    cnt = np.maximum(cnt, 1.0)
    NTP = c.NT * 128
    cma = cm[:, 0, :]
    i47 = np.eye(c.C, dtype=np.float32)
    ident = np.eye(128, dtype=np.float32)

    in_maps = []
    for ci in range(c.ncores):
        r0 = ci * c.NS
        xs = x[r0:r0 + c.NS]                      # [NS, H]
        lab = label[r0:r0 + c.NS]
        ps = p[r0:r0 + c.NS]                      # [NS, C]
        d_t = np.zeros((NTP, c.C), np.float32)
        d_t[np.arange(c.NS), lab] = 1.0 / cnt[lab]
        p_pad = np.zeros((NTP, c.C), np.float32)
        p_pad[:c.NS] = ps
        in_maps.append({
            "xin_t": np.ascontiguousarray(xs.T),
            "d_t": np.ascontiguousarray(d_t.reshape(c.NT, 128, c.C)),
            "p_t": np.ascontiguousarray(
                p_pad.reshape(c.NT, 128, c.C).transpose(0, 2, 1)),
            "gidx": gidx[ci], "sidx": sidx[ci], "wgt": wgt[ci],
            "lin0w": lin0_w, "lin0b": lin0_b.reshape(-1, 1),
            "lin1w": lin1_w, "lin1b": lin1_b.reshape(-1, 1),
            "convw": conv_w, "cma": cma,
            "cmat": np.ascontiguousarray(cma.T),
            "i47": i47, "ident": ident,
        })
    return in_maps, runs


_BUILT = {}


def kernel(**inputs):
    cfg = DEF
    global RUNS
    in_maps, runs = _prep_inputs(cfg, inputs)
    key = "default"
    if key not in _BUILT:
        RUNS = runs
        _BUILT[key] = build_nc(cfg)
    nc = _BUILT[key]
    res = bass_utils.run_bass_kernel_spmd(nc, in_maps,
                                          core_ids=list(range(cfg.ncores)))
    outs = [res.results[ci]["out_t"].T for ci in range(cfg.ncores)]
    return np.ascontiguousarray(np.concatenate(outs, 0))



# revision 9
# speedup vs baseline: 2.6293x; 2.6293x over previous
"""GCN2 (nn_GCN2_42331197669873) Bass kernel for 8 TRN2 NeuronCores.

Strategy: graph/data parallel, nodes sharded row-wise (12544 padded rows
per core).  The sparse propagate is a one-hot-matmul segment sum: edge
source rows are gathered (SWDGE dma_gather spread over 4 queues = 4 Q7
core pairs, ~3.1 ns/token vs 8.7 single-queue) as fp16 rows into SBUF
token groups of 128; each group is multiplied on the tensor engine by an
on-the-fly selection matrix S[tok, dst] = w * (dst_off(tok)==col),
accumulating into a per-supertile PSUM bank [H, 512].  No scatter-add,
no HBM aggregation buffers, f32 accumulation.  The dense combine
(class-center term, 0.45x + 0.1x0, beta-folded conv) continues in the
same PSUM bank via matmul injections, so there is one PSUM round trip
per tile per layer.

All x-valued tensors are fp16: the class centers are nearly identical
(||c|| ~ 28x the pairwise distances), so the normalized class-difference
term amplifies center errors ~8x and bf16 node features are not accurate
enough.  fp16 also keeps the S-build is_equal compare exact (integers
<= 2048).

Node features are replicated per layer with two half AllGathers
(double-buffered x_rep so the next layer's AG overlaps this layer's
tail).  Weights/class tensors are tiny and resident.

kernel(**inputs) takes the FULL unsharded inputs and returns the FULL
[100000, 47] output; sharding + preprocessing happens on host inside.
"""

import numpy as np
import ml_dtypes

from concourse import bass, bacc, tile, mybir, bass_utils
from concourse import library_config
from concourse.mybir import AxisListType
import concourse.tile_sem_assignment as _tsa
from concourse import bass_isa as _bisa

NQUEUES = 4
ACT_MOD = 0          # every ACT_MOD-th S-build goes to the ACT engine (0=off)

# Tile round-robins Pool-engine DMAs over all DMASW sem lanes ignoring
# queue_num; mixing SWDGE queues on one lane breaks its in-order-completion
# assumption (sim: "sem locked to SWDGE queue"). Segregate lanes by queue:
# queue q -> lanes [2q, 2q+1].
_orig_assign_tick = _tsa.TileClockTick._assign_tick


def _assign_tick_qsplit(self, inst):
    if (isinstance(inst, _tsa.DMAInst)
            and inst.engine == mybir.EngineType.Pool
            and not isinstance(inst, _bisa.UserSyncedRemoteDMADescs)
            and self.swdge_sem_count >= NQUEUES * 2):
        qn = getattr(inst, "queue_num", 0) or 0
        lanes = self.swdge_sem_count // NQUEUES
        if not hasattr(self, "_qrr"):
            self._qrr = {}
        r = self._qrr.get(qn, 0)
        self._qrr[qn] = r + 1
        self.next_sw_dma_idx = (qn % NQUEUES) * lanes + r % lanes
    return _orig_assign_tick(self, inst)


_tsa.TileClockTick._assign_tick = _assign_tick_qsplit

F32 = mybir.dt.float32
BF16 = mybir.dt.bfloat16
FP16 = mybir.dt.float16
I16 = mybir.dt.int16
AF = mybir.ActivationFunctionType


class Cfg:
    def __init__(self):
        self.N, self.E, self.C, self.H = 100000, 800000, 47, 128
        self.ncores = 8
        self.NS = self.N // self.ncores      # real nodes per core
        self.NT = 98                         # 128-row tiles per core
        self.NSP = self.NT * 128             # padded nodes per core
        self.HT = 49                         # tiles per half
        self.HR = self.HT * 128              # rows per half shard (6272)
        self.REG = self.ncores * self.HR     # x_rep region rows (50176)
        self.W = self.REG // 2               # gather window rows (25088)
        self.NW = 4                          # windows (2 per region)
        self.STW = 512                       # supertile width (psum bank)
        self.NST = (self.NSP + 511) // 512   # supertiles per core (25)
        self.CHST = 4                        # supertiles per chunk
        self.NCH = (self.NST + 3) // 4       # chunks (7)
        self.L = 4
        self.alpha, self.theta, self.rsl = 0.1, 0.5, 0.5


DEF = Cfg()


# ----------------------------------------------------------------------
# host-side edge preprocessing
# ----------------------------------------------------------------------

def _prep_edges(cfg, edge_index, edge_weight):
    """Token layout: cells (supertile st, window w), slots = 128-multiple
    max-over-cores capacity.  Within a cell tokens sorted by gather idx.

    Returns gidx [nc,128,COLS] i16, dstv/wv [nc,128,NB] f32,
    slots [NST][NW] (# 128-token groups per cell, shared across cores).
    """
    c = cfg
    src = np.asarray(edge_index[0], np.int64)
    dst = np.asarray(edge_index[1], np.int64)
    w = np.asarray(edge_weight, np.float32) * (1.0 - c.rsl) * (1.0 - c.alpha)
    nc_, NS = c.ncores, c.NS

    core = dst // NS
    r = dst - core * NS
    st = r // c.STW
    doff = r - st * c.STW
    cs, rs = src // NS, src % NS
    ts, ps = rs // 128, rs % 128
    reg = (ts >= c.HT).astype(np.int64)
    row = cs * c.HR + ps * c.HT + (ts - c.HT * reg)
    win = 2 * reg + row // c.W
    gix = row - (row // c.W) * c.W

    cnt = np.zeros((nc_, c.NST, c.NW), np.int64)
    np.add.at(cnt, (core, st, win), 1)
    cap = cnt.max(axis=0)
    slots = (cap + 127) // 128                      # groups per cell

    nb = int(slots.sum())                           # total groups
    TOK = nb * 128
    gidx = np.zeros((nc_, 128, TOK // 16), np.int16)
    dstv = np.full((nc_, 128, nb), -1.0, np.float32)
    wv = np.zeros((nc_, 128, nb), np.float32)

    cell_g0 = np.zeros((c.NST, c.NW), np.int64)
    g = 0
    for s in range(c.NST):
        for b in range(c.NW):
            cell_g0[s, b] = g
            g += slots[s, b]

    ti = np.arange(TOK)
    rows16 = (ti % 16)[None, :] + 16 * np.arange(8)[:, None]   # [8, TOK]
    cols16 = ti // 16

    key = (core * c.NST + st) * c.NW + win
    order = np.lexsort((gix, key))
    ksort = key[order]
    bounds = np.searchsorted(ksort, np.arange(nc_ * c.NST * c.NW + 1))
    for ci in range(nc_):
        g_lin = np.zeros(TOK, np.int16)
        d_lin = np.full(TOK, -1.0, np.float32)
        w_lin = np.zeros(TOK, np.float32)
        for s in range(c.NST):
            for b in range(c.NW):
                kk = (ci * c.NST + s) * c.NW + b
                sel = order[bounds[kk]:bounds[kk + 1]]
                n = len(sel)
                t0 = int(cell_g0[s, b]) * 128
                g_lin[t0:t0 + n] = gix[sel].astype(np.int16)
                d_lin[t0:t0 + n] = doff[sel].astype(np.float32)
                w_lin[t0:t0 + n] = w[sel]
        for gg in range(8):
            gidx[ci, rows16[gg], cols16] = g_lin
        dstv[ci, ti % 128, ti // 128] = d_lin
        wv[ci, ti % 128, ti // 128] = w_lin
    return gidx, dstv, wv, slots


# ----------------------------------------------------------------------
# device program
# ----------------------------------------------------------------------

def build_nc(cfg, slots):
    c = cfg
    nc = bacc.Bacc(None, target_bir_lowering=False, debug=False,
                   num_swdge_queues=NQUEUES)
    NT, NSP, C_, H = c.NT, c.NSP, c.C, c.H
    L = c.L
    NB = int(slots.sum())
    TOK = NB * 128

    def dram_in(name, shape, dt=F32):
        return nc.declare_dram_parameter(name, shape, dt, isOutput=False)

    xin_T = dram_in("xin_T", [H, NT, 128], FP16)
    d_T = dram_in("d_T", [128, NT, C_], F32)
    p_T = dram_in("p_T", [C_, NSP], FP16)
    gidx = dram_in("gidx", [128, TOK // 16], I16)
    dstv = dram_in("dstv", [128, NB], F32)
    wv = dram_in("wv", [128, NB], F32)
    lin0w = dram_in("lin0w", [H, H])
    lin0b = dram_in("lin0b", [H, 1])
    lin1w = dram_in("lin1w", [H, C_])
    lin1b = dram_in("lin1b", [C_, 1])
    wceff = dram_in("wceff", [L, H, H])
    cma45 = dram_in("cma45", [C_, C_])
    cmat45 = dram_in("cmat45", [C_, C_])
    i47 = dram_in("i47", [C_, C_])
    ident = dram_in("ident", [128, 128])
    iota = dram_in("iota", [128, c.STW], FP16)
    out_T = nc.declare_dram_parameter("out_T", [C_, NSP], F32, isOutput=True)

    # internal DRAM
    x_rep = [[nc.dram_tensor(f"x_rep{h}_{pbuf}", [c.REG, H], FP16,
                             addr_space="Shared")
              for pbuf in range(2)] for h in range(2)]
    sh = [[nc.dram_tensor(f"sh{i}{'AB'[h]}", [128, c.HT, H], FP16)
           for h in range(2)] for i in range(3)]
    # feature-major fp16 x buffers; xTh[0] holds x0 and is never overwritten
    xTh = [nc.dram_tensor(f"xTh{i}", [128, NT, H], FP16) for i in range(3)]
    cen_in = nc.dram_tensor("cen_in", [C_, H], F32)
    cen_out = nc.dram_tensor("cen_out", [C_, H], F32, addr_space="Shared")

    rg = [list(range(c.ncores))]
    CURS = [0, 1, 2, 1]
    NXTS = [1, 2, 1, 2]

    cell_g0 = np.zeros((c.NST, c.NW), np.int64)
    g = 0
    for s in range(c.NST):
        for b in range(c.NW):
            cell_g0[s, b] = g
            g += slots[s, b]

    with tile.TileContext(nc) as tc:
        nc.gpsimd.load_library(library_config.mlp)
        with (
            tc.tile_pool(name="const", bufs=1) as cpool,
            tc.tile_pool(name="edge", bufs=1) as epool,
            tc.tile_pool(name="bslab", bufs=2) as bpool,
            tc.tile_pool(name="oslab", bufs=2) as opool,
            tc.tile_pool(name="gt", bufs=8) as gpool,
            tc.tile_pool(name="smat", bufs=6) as spool,
            tc.tile_pool(name="sb", bufs=4) as pool,
            tc.tile_pool(name="bank", bufs=c.CHST, space="PSUM") as bankp,
            tc.tile_pool(name="ps2", bufs=2, space="PSUM") as ps2p,
            tc.tile_pool(name="psn", bufs=1, space="PSUM") as psnp,
            tc.tile_pool(name="psA", bufs=1, space="PSUM") as psAp,
        ):
            # ---- resident constants ----
            lin0w_sb = cpool.tile([H, H], F32)
            nc.sync.dma_start(lin0w_sb[:], lin0w[:, :])
            lin0w_h = cpool.tile([H, H], FP16)
            nc.vector.tensor_copy(lin0w_h[:], lin0w_sb[:])
            lin0b_sb = cpool.tile([H, 1], F32)
            nc.sync.dma_start(lin0b_sb[:], lin0b[:, :])
            lin1w_sb = cpool.tile([H, C_], F32)
            nc.sync.dma_start(lin1w_sb[:], lin1w[:, :])
            lin1w_h = cpool.tile([H, C_], FP16)
            nc.vector.tensor_copy(lin1w_h[:], lin1w_sb[:])
            lin1b_sb = cpool.tile([C_, 1], F32)
            nc.sync.dma_start(lin1b_sb[:], lin1b[:, :])
            wceff_sb = cpool.tile([H, L * H], F32)
            for i in range(L):
                nc.sync.dma_start(wceff_sb[:, i * H:(i + 1) * H], wceff[i])
            cma_sb = cpool.tile([C_, C_], F32)
            nc.sync.dma_start(cma_sb[:], cma45[:, :])
            cmat_sb = cpool.tile([C_, C_], F32)
            nc.sync.dma_start(cmat_sb[:], cmat45[:, :])
            i47_sb = cpool.tile([C_, C_], F32)
            nc.sync.dma_start(i47_sb[:], i47[:, :])
            ident_sb = cpool.tile([128, 128], F32)
            nc.sync.dma_start(ident_sb[:], ident[:, :])
            identh_sb = cpool.tile([128, 128], FP16)
            nc.vector.tensor_copy(identh_sb[:], ident_sb[:])
            i45h_sb = cpool.tile([128, 128], FP16)
            nc.vector.tensor_scalar(i45h_sb[:], ident_sb[:], 0.45, None,
                                    mybir.AluOpType.mult)
            i10h_sb = cpool.tile([128, 128], FP16)
            nc.vector.tensor_scalar(i10h_sb[:], ident_sb[:], 0.1, None,
                                    mybir.AluOpType.mult)
            iota_sb = cpool.tile([128, c.STW], FP16)
            nc.sync.dma_start(iota_sb[:], iota[:, :])

            # ---- resident edge data ----
            gi_sb = epool.tile([128, TOK // 16], I16)
            nc.sync.dma_start(gi_sb[:], gidx[:, :])
            dv_sb = epool.tile([128, NB], F32)
            nc.sync.dma_start(dv_sb[:], dstv[:, :])
            wv_sb = epool.tile([128, NB], F32)
            nc.sync.dma_start(wv_sb[:], wv[:, :])
            d_res = epool.tile([128, NT, C_], F32)
            nc.sync.dma_start(d_res[:], d_T[:, :, :])
            cen0_sb = epool.tile([C_, H], F32)
            if ACT_MOD:
                # aux for ACT-engine S-build: t=Square((iota-d)/32),
                # S=Relu(w - 4096*w*t)
                dvn_sb = epool.tile([128, NB], F32)
                nc.vector.tensor_scalar(dvn_sb[:], dv_sb[:], -1.0 / 32, None,
                                        mybir.AluOpType.mult)
                wvn_sb = epool.tile([128, NB], F32)
                nc.vector.tensor_scalar(wvn_sb[:], wv_sb[:], -4096.0, None,
                                        mybir.AluOpType.mult)

            CHT = c.CHST * 4                       # tiles per chunk (16)

            def chunk_tiles(ch):
                t0 = ch * CHT
                return t0, min(CHT, NT - t0)

            def write_sh(dst_sh, shs, t0, ntl):
                if t0 + ntl <= c.HT:
                    nc.sync.dma_start(dst_sh[0][:, t0:t0 + ntl, :],
                                      shs[:, :ntl, :])
                elif t0 >= c.HT:
                    nc.sync.dma_start(
                        dst_sh[1][:, t0 - c.HT:t0 - c.HT + ntl, :],
                        shs[:, :ntl, :])
                else:
                    n1 = c.HT - t0
                    nc.sync.dma_start(dst_sh[0][:, t0:c.HT, :],
                                      shs[:, :n1, :])
                    nc.sync.dma_start(dst_sh[1][:, 0:ntl - n1, :],
                                      shs[:, n1:ntl, :])

            # ---- lin0 (also accumulates psA0 = d^T x0 in f32) ----
            psA = psAp.tile([C_, H], F32, tag="cen")
            for ch in range(c.NCH):
                t0, ntl = chunk_tiles(ch)
                xi = bpool.tile([128, CHT, H], FP16, tag="xTs")
                nc.sync.dma_start(xi[:, :ntl, :], xin_T[:, t0:t0 + ntl, :])
                xhs = opool.tile([128, CHT, H], FP16, tag="xhs")
                shs = opool.tile([128, CHT, H], FP16, tag="shs")
                for j in range(ntl):
                    t = t0 + j
                    ps0 = ps2p.tile([H, 128], F32, tag="b")
                    nc.tensor.matmul(ps0[:], lin0w_h[:], xi[:, j, :],
                                     start=True, stop=True)
                    nc.scalar.activation(xhs[:, j, :], ps0[:], AF.Relu,
                                         bias=lin0b_sb[:, 0:1])
                    xf = pool.tile([H, 128], F32, tag="xf")
                    nc.scalar.activation(xf[:], ps0[:], AF.Relu,
                                         bias=lin0b_sb[:, 0:1])
                    psn = psnp.tile([128, 128], F32, tag="bb")
                    nc.tensor.transpose(psn[:], xf[:], ident_sb[:])
                    nc.vector.tensor_copy(shs[:, j, :], psn[:])
                    sf = pool.tile([128, H], F32, tag="sf")
                    nc.vector.tensor_copy(sf[:], psn[:])
                    nc.tensor.matmul(psA[:], d_res[:, t, :], sf[:],
                                     start=(t == 0), stop=(t == NT - 1))
                nc.sync.dma_start(xTh[0][:, t0:t0 + ntl, :], xhs[:, :ntl, :])
                write_sh(sh[0], shs, t0, ntl)
            nc.vector.tensor_copy(cen0_sb[:], psA[:])

            # ---- layers ----
            qrr = [0]

            for li in range(L):
                cur, nxt = CURS[li], NXTS[li]
                pb = li % 2
                for h in range(2):
                    nc.gpsimd.collective_compute(
                        "AllGather", mybir.AluOpType.bypass,
                        replica_groups=rg,
                        ins=[sh[cur][h].ap().opt()],
                        outs=[x_rep[h][pb].ap().opt()],
                    )

                cenp = pool.tile([C_, H], F32, tag="cenp")
                if li == 0:
                    nc.vector.tensor_scalar(cenp[:], cen0_sb[:], 1.1, None,
                                            mybir.AluOpType.mult)
                else:
                    nc.vector.tensor_scalar(cenp[:], cen0_sb[:], 0.1, None,
                                            mybir.AluOpType.mult)
                    nc.vector.tensor_add(cenp[:], cenp[:], psA[:])
                nc.sync.dma_start(cen_in[:, :], cenp[:])
                nc.gpsimd.collective_compute(
                    "AllReduce", mybir.AluOpType.add, replica_groups=rg,
                    ins=[cen_in.ap().opt()], outs=[cen_out.ap().opt()],
                )
                cen = pool.tile([C_, H], F32, tag="cen_sb")
                nc.sync.dma_start(cen[:], cen_out[:, :])

                # - r_cls from centers (Gram trick), cma pre-scaled 0.45 -
                pst = ps2p.tile([128, C_], F32, tag="b")
                nc.tensor.transpose(pst[:, :], cen[:], ident_sb[:C_, :C_])
                cT = pool.tile([128, C_], F32, tag="cT")
                nc.vector.tensor_copy(cT[:], pst[:, :])
                psg = ps2p.tile([C_, C_], F32, tag="b")
                nc.tensor.matmul(psg[:], cT[:], cT[:], start=True, stop=True)
                gg = pool.tile([C_, C_], F32, tag="gg")
                nc.vector.tensor_copy(gg[:], psg[:])
                gd = pool.tile([C_, C_], F32, tag="gd")
                nc.vector.tensor_mul(gd[:], gg[:], i47_sb[:])
                n2 = pool.tile([C_, 1], F32, tag="n2")
                nc.vector.reduce_sum(n2[:], gd[:], AxisListType.X)
                t1 = pool.tile([C_, C_], F32, tag="t1")
                nc.vector.tensor_scalar(t1[:], gg[:], -1.0, n2[:, 0:1],
                                        mybir.AluOpType.mult,
                                        mybir.AluOpType.add)
                ps1 = ps2p.tile([C_, C_], F32, tag="b")
                nc.tensor.transpose(ps1[:], t1[:], ident_sb[:C_, :C_])
                nrm = pool.tile([C_, C_], F32, tag="nrm")
                nc.vector.tensor_add(nrm[:], t1[:], ps1[:])
                nc.vector.tensor_relu(nrm[:], nrm[:])
                nc.vector.tensor_add(nrm[:], nrm[:], i47_sb[:])
                rn = pool.tile([C_, C_], F32, tag="rn")
                nc.scalar.sqrt(rn[:], nrm[:])
                inv = pool.tile([C_, C_], F32, tag="inv")
                nc.vector.reciprocal(inv[:], rn[:])
                amat = pool.tile([C_, C_], F32, tag="amat")
                nc.vector.tensor_mul(amat[:], cma_sb[:], inv[:])
                atm = pool.tile([C_, C_], F32, tag="atm")
                nc.vector.tensor_mul(atm[:], cmat_sb[:], inv[:])
                rs = pool.tile([C_, 1], F32, tag="rs")
                nc.vector.reduce_sum(rs[:], amat[:], AxisListType.X)
                psm = ps2p.tile([C_, H], F32, tag="b")
                nc.tensor.matmul(psm[:], atm[:], cen[:], start=True, stop=True)
                rcls = pool.tile([C_, H], F32, tag="rcls")
                nc.vector.tensor_scalar(rcls[:], cen[:], rs[:, 0:1], None,
                                        mybir.AluOpType.mult)
                nc.vector.tensor_sub(rcls[:], rcls[:], psm[:])
                rclsh = pool.tile([C_, H], FP16, tag="rclsh")
                nc.vector.tensor_copy(rclsh[:], rcls[:])

                # - propagate + pass B, chunked -
                for ch in range(c.NCH):
                    t0, ntl = chunk_tiles(ch)
                    st0 = ch * c.CHST
                    nst = min(c.CHST, c.NST - st0)
                    xTs = bpool.tile([128, CHT, H], FP16, tag="xTs")
                    nc.sync.dma_start(xTs[:, :ntl, :],
                                      xTh[cur][:, t0:t0 + ntl, :])
                    x0s2 = bpool.tile([128, CHT, H], FP16, tag="x0s2")
                    nc.sync.dma_start(x0s2[:, :ntl, :],
                                      xTh[0][:, t0:t0 + ntl, :])
                    pts = bpool.tile([C_, CHT * 128], FP16, tag="pts")
                    nc.sync.dma_start(pts[:, :ntl * 128],
                                      p_T[:, t0 * 128:(t0 + ntl) * 128])

                    banks = []
                    for si in range(nst):
                        st = st0 + si
                        bank = bankp.tile([H, c.STW], F32, tag="bank")
                        banks.append(bank)
                        first = True
                        for wnd in range(c.NW):
                            g0, ng = int(cell_g0[st, wnd]), int(slots[st, wnd])
                            if ng == 0:
                                continue
                            rep = x_rep[wnd // 2][pb]
                            base = (wnd % 2) * c.W
                            gts = []
                            for q0 in range(0, ng, 8):
                                q1 = min(q0 + 8, ng)
                                ntok = (q1 - q0) * 128
                                gt = gpool.tile([128, 8, H], FP16, tag="g")
                                nc.gpsimd.dma_gather(
                                    gt[:, :q1 - q0, :],
                                    rep[base:base + c.W, :],
                                    gi_sb[:, (g0 + q0) * 8:(g0 + q1) * 8],
                                    num_idxs=ntok, num_idxs_reg=ntok,
                                    elem_size=H,
                                    queue_num=qrr[0] % NQUEUES,
                                )
                                qrr[0] += 1
                                gts.append((gt, q0, q1 - q0))
                            for (gt, q0, nq) in gts:
                                for k in range(nq):
                                    gl = g0 + q0 + k
                                    S = spool.tile([128, c.STW], FP16,
                                                   tag="S")
                                    if ACT_MOD and gl % ACT_MOD == 0:
                                        St = spool.tile([128, c.STW], FP16,
                                                        tag="St")
                                        nc.scalar.activation(
                                            St[:], iota_sb[:], AF.Square,
                                            bias=dvn_sb[:, gl:gl + 1],
                                            scale=1.0 / 32)
                                        nc.scalar.activation(
                                            S[:], St[:], AF.Relu,
                                            bias=wv_sb[:, gl:gl + 1],
                                            scale=wvn_sb[:, gl:gl + 1])
                                    else:
                                        nc.vector.tensor_scalar(
                                            S[:], iota_sb[:],
                                            dv_sb[:, gl:gl + 1],
                                            wv_sb[:, gl:gl + 1],
                                            mybir.AluOpType.is_equal,
                                            mybir.AluOpType.mult)
                                    nc.tensor.matmul(
                                        bank[:], gt[:, k, :], S[:],
                                        start=first, stop=False)
                                    first = False
                        for j in range(4):
                            t = st * 4 + j
                            if t >= NT:
                                break
                            jj = t - t0
                            sl = bank[:, j * 128:(j + 1) * 128]
                            nc.tensor.matmul(sl, rclsh[:],
                                             pts[:, jj * 128:(jj + 1) * 128],
                                             start=first, stop=False)
                            nc.tensor.matmul(sl, i45h_sb[:], xTs[:, jj, :],
                                             start=False, stop=False)
                            nc.tensor.matmul(sl, i10h_sb[:], x0s2[:, jj, :],
                                             start=False, stop=True)
                            first = False

                    # pass B compute per tile (also accumulates next
                    # layer's centers from f32 data)
                    if ch == 0 and li < L - 1:
                        psA = psAp.tile([C_, H], F32, tag="cen")
                    xhs = opool.tile([128, CHT, H], FP16, tag="xhs")
                    shs = opool.tile([128, CHT, H], FP16, tag="shs")
                    for si in range(nst):
                        st = st0 + si
                        for j in range(4):
                            t = st * 4 + j
                            if t >= NT:
                                break
                            jj = t - t0
                            u = pool.tile([H, 128], F32, tag="u")
                            nc.vector.tensor_copy(
                                u[:], banks[si][:, j * 128:(j + 1) * 128])
                            ps2 = ps2p.tile([H, 128], F32, tag="b")
                            nc.tensor.matmul(ps2[:],
                                             wceff_sb[:, li * H:(li + 1) * H],
                                             u[:], start=True, stop=True)
                            nc.scalar.activation(xhs[:, jj, :], ps2[:],
                                                 AF.Relu)
                            xf = pool.tile([H, 128], F32, tag="xf")
                            nc.scalar.activation(xf[:], ps2[:], AF.Relu)
                            psn = psnp.tile([128, 128], F32, tag="bb")
                            nc.tensor.transpose(psn[:], xf[:], ident_sb[:])
                            nc.vector.tensor_copy(shs[:, jj, :], psn[:])
                            if li < L - 1:
                                sf = pool.tile([128, H], F32, tag="sf")
                                nc.vector.tensor_copy(sf[:], psn[:])
                                nc.tensor.matmul(psA[:], d_res[:, t, :],
                                                 sf[:], start=(t == 0),
                                                 stop=(t == NT - 1))
                    nc.sync.dma_start(xTh[nxt][:, t0:t0 + ntl, :],
                                      xhs[:, :ntl, :])
                    write_sh(sh[nxt], shs, t0, ntl)

            # ---- lin1 ----
            fin = NXTS[L - 1]
            for ch in range(c.NCH):
                t0, ntl = chunk_tiles(ch)
                xi = bpool.tile([128, CHT, H], FP16, tag="xTs")
                nc.sync.dma_start(xi[:, :ntl, :], xTh[fin][:, t0:t0 + ntl, :])
                ots = opool.tile([C_, CHT * 128], F32, tag="ots", bufs=1)
                for j in range(ntl):
                    psf = ps2p.tile([C_, 128], F32, tag="b")
                    nc.tensor.matmul(psf[:], lin1w_h[:], xi[:, j, :],
                                     start=True, stop=True)
                    nc.vector.tensor_scalar(ots[:, j * 128:(j + 1) * 128],
                                            psf[:], lin1b_sb[:, 0:1], None,
                                            mybir.AluOpType.add)
                nc.sync.dma_start(out_T[:, t0 * 128:(t0 + ntl) * 128],
                                  ots[:, :ntl * 128])

    nc.compile()
    return nc


def _load_sh_slab(nc, c, shp, dest, t0, ntl):
    """Load node-major tiles [t0, t0+ntl) from half-split sh into dest."""
    if t0 + ntl <= c.HT:
        nc.sync.dma_start(dest[:, :ntl, :], shp[0][:, t0:t0 + ntl, :])
    elif t0 >= c.HT:
        nc.sync.dma_start(dest[:, :ntl, :],
                          shp[1][:, t0 - c.HT:t0 - c.HT + ntl, :])
    else:
        n1 = c.HT - t0
        nc.sync.dma_start(dest[:, :n1, :], shp[0][:, t0:c.HT, :])
        nc.sync.dma_start(dest[:, n1:ntl, :], shp[1][:, 0:ntl - n1, :])


# ----------------------------------------------------------------------
# host wrapper
# ----------------------------------------------------------------------

def _prep_inputs(cfg, inputs):
    c = cfg
    x = np.asarray(inputs["x"], np.float32)
    label = np.asarray(inputs["label"], np.int64)
    p = np.asarray(inputs["p"], np.float32)
    cm = np.asarray(inputs["cm"], np.float32)
    lin0_w = np.asarray(inputs["lin0_w"], np.float32)
    lin0_b = np.asarray(inputs["lin0_b"], np.float32)
    lin1_w = np.asarray(inputs["lin1_w"], np.float32)
    lin1_b = np.asarray(inputs["lin1_b"], np.float32)
    conv_w = np.asarray(inputs["conv_w"], np.float32)

    gidx, dstv, wv, slots = _prep_edges(cfg, inputs["edge_index"],
                                        inputs["edge_weight"])

    cnt = np.bincount(label, minlength=c.C).astype(np.float32)
    cnt = np.maximum(cnt, 1.0)
    cma = cm[:, 0, :] * (c.rsl * (1.0 - c.alpha))      # 0.45 fold
    i47 = np.eye(c.C, dtype=np.float32)
    ident = np.eye(128, dtype=np.float32)
    iota = np.tile(np.arange(c.STW, dtype=np.float16)[None, :], (128, 1))
    wceff = np.zeros((c.L, c.H, c.H), np.float32)
    for i in range(c.L):
        beta = float(np.log(c.theta / (i + 1) + 1.0))
        wceff[i] = (1.0 - beta) * np.eye(c.H, dtype=np.float32) \
            + beta * conv_w[i]

    in_maps = []
    for ci in range(c.ncores):
        r0 = ci * c.NS
        xs = np.zeros((c.NSP, c.H), np.float32)
        xs[:c.NS] = x[r0:r0 + c.NS]
        lab = label[r0:r0 + c.NS]
        d_t = np.zeros((c.NSP, c.C), np.float32)
        d_t[np.arange(c.NS), lab] = 1.0 / cnt[lab]
        p_pad = np.zeros((c.NSP, c.C), np.float32)
        p_pad[:c.NS] = p[r0:r0 + c.NS]
        in_maps.append({
            "xin_T": np.ascontiguousarray(xs.T).reshape(
                c.H, c.NT, 128).astype(np.float16),
            "d_T": np.ascontiguousarray(
                d_t.reshape(c.NT, 128, c.C).transpose(1, 0, 2)),
            "p_T": np.ascontiguousarray(p_pad.T).astype(np.float16),
            "gidx": gidx[ci], "dstv": dstv[ci], "wv": wv[ci],
            "lin0w": lin0_w, "lin0b": lin0_b.reshape(-1, 1),
            "lin1w": lin1_w, "lin1b": lin1_b.reshape(-1, 1),
            "wceff": wceff, "cma45": cma,
            "cmat45": np.ascontiguousarray(cma.T),
            "i47": i47, "ident": ident, "iota": iota,
        })
    return in_maps, slots


_BUILT = {}


def kernel(**inputs):
    cfg = DEF
    in_maps, slots = _prep_inputs(cfg, inputs)
    key = "default"
    if key not in _BUILT:
        _BUILT[key] = build_nc(cfg, slots)
    nc = _BUILT[key]
    res = bass_utils.run_bass_kernel_spmd(nc, in_maps,
                                          core_ids=list(range(cfg.ncores)))
    outs = [res.results[ci]["out_T"].T[:cfg.NS] for ci in range(cfg.ncores)]
    return np.ascontiguousarray(np.concatenate(outs, 0).astype(np.float32))


# revision 14
# speedup vs baseline: 3.0377x; 1.1553x over previous
"""GCN2 (nn_GCN2_42331197669873) Bass kernel for 8 TRN2 NeuronCores.

Strategy: graph/data parallel, nodes sharded row-wise (12544 padded rows
per core).  The sparse propagate is a one-hot-matmul segment sum: edge
source rows are gathered (SWDGE dma_gather spread over 4 queues = 4 Q7
core pairs, ~3.1 ns/token vs 8.7 single-queue) as fp16 rows into SBUF
token groups of 128; each group is multiplied on the tensor engine by an
on-the-fly selection matrix S[tok, dst] = w * (dst_off(tok)==col),
accumulating into a per-supertile PSUM bank [H, 512].  No scatter-add,
no HBM aggregation buffers, f32 accumulation.  The dense combine
(class-center term, 0.45x + 0.1x0, beta-folded conv) continues in the
same PSUM bank via matmul injections, so there is one PSUM round trip
per tile per layer.

All x-valued tensors are fp16: the class centers are nearly identical
(||c|| ~ 28x the pairwise distances), so the normalized class-difference
term amplifies center errors ~8x and bf16 node features are not accurate
enough.  fp16 also keeps the S-build is_equal compare exact (integers
<= 2048).

Node features are replicated per layer with two half AllGathers
(double-buffered x_rep so the next layer's AG overlaps this layer's
tail).  Weights/class tensors are tiny and resident.

kernel(**inputs) takes the FULL unsharded inputs and returns the FULL
[100000, 47] output; sharding + preprocessing happens on host inside.
"""

import numpy as np
import ml_dtypes

from concourse import bass, bacc, tile, mybir, bass_utils
from concourse import library_config
from concourse.mybir import AxisListType
import concourse.tile_sem_assignment as _tsa
from concourse import bass_isa as _bisa

NQUEUES = 4
ACT_MOD = 0          # every ACT_MOD-th S-build goes to the ACT engine (0=off)

# Tile round-robins Pool-engine DMAs over all DMASW sem lanes ignoring
# queue_num; mixing SWDGE queues on one lane breaks its in-order-completion
# assumption (sim: "sem locked to SWDGE queue"). Segregate lanes by queue:
# queue q -> lanes [2q, 2q+1].
_orig_assign_tick = _tsa.TileClockTick._assign_tick


def _assign_tick_qsplit(self, inst):
    if (isinstance(inst, _tsa.DMAInst)
            and inst.engine == mybir.EngineType.Pool
            and not isinstance(inst, _bisa.UserSyncedRemoteDMADescs)
            and self.swdge_sem_count >= NQUEUES * 2):
        qn = getattr(inst, "queue_num", 0) or 0
        lanes = self.swdge_sem_count // NQUEUES
        if not hasattr(self, "_qrr"):
            self._qrr = {}
        r = self._qrr.get(qn, 0)
        self._qrr[qn] = r + 1
        self.next_sw_dma_idx = (qn % NQUEUES) * lanes + r % lanes
    return _orig_assign_tick(self, inst)


_tsa.TileClockTick._assign_tick = _assign_tick_qsplit

F32 = mybir.dt.float32
BF16 = mybir.dt.bfloat16
FP16 = mybir.dt.float16
I16 = mybir.dt.int16
AF = mybir.ActivationFunctionType


class Cfg:
    def __init__(self):
        self.N, self.E, self.C, self.H = 100000, 800000, 47, 128
        self.ncores = 8
        self.NS = self.N // self.ncores      # real nodes per core
        self.NT = 98                         # 128-row tiles per core
        self.NSP = self.NT * 128             # padded nodes per core
        self.HT = 49                         # tiles per half
        self.HR = self.HT * 128              # rows per half shard (6272)
        self.REG = self.ncores * self.HR     # x_rep region rows (50176)
        self.W = self.REG // 2               # gather window rows (25088)
        self.NW = 4                          # windows (2 per region)
        self.STW = 512                       # supertile width (psum bank)
        self.NST = (self.NSP + 511) // 512   # supertiles per core (25)
        self.NRES = 2                        # node interleave classes
        self.SW = self.STW // self.NRES      # S matrix width (256)
        self.CHST = 4                        # supertiles per chunk
        self.NCH = (self.NST + 3) // 4       # chunks (7)
        self.L = 4
        self.alpha, self.theta, self.rsl = 0.1, 0.5, 0.5


DEF = Cfg()


# ----------------------------------------------------------------------
# host-side edge preprocessing
# ----------------------------------------------------------------------

def _node_perm(cfg):
    """Physical position of each padded local node index (within-supertile
    interleave by residue class, so S matrices are STW/NRES wide)."""
    c = cfg
    r = np.arange(c.NSP)
    st = r // c.STW
    stw = np.minimum(c.STW, c.NSP - st * c.STW)
    sw = stw // c.NRES
    w = r - st * c.STW
    return st * c.STW + (w % c.NRES) * sw + w // c.NRES


def _prep_edges(cfg, edge_index, edge_weight):
    """Token layout: cells (supertile st, residue rr, window w); slots =
    128-multiple max-over-cores capacity; tokens sorted by gather idx,
    trailing pads use idx -1 (trimmed by the Q7 per core).

    Returns gidx [nc,128,COLS] i16, dstv/wv [nc,128,NB] f32,
    slots [NST][NRES][NW].
    """
    c = cfg
    perm = _node_perm(c)
    src = np.asarray(edge_index[0], np.int64)
    dst = np.asarray(edge_index[1], np.int64)
    w = np.asarray(edge_weight, np.float32) * (1.0 - c.rsl) * (1.0 - c.alpha)
    nc_, NS = c.ncores, c.NS

    core = dst // NS
    r = dst - core * NS
    st = r // c.STW
    stw = np.minimum(c.STW, c.NSP - st * c.STW)
    sw = stw // c.NRES
    within = r - st * c.STW
    rr = within % c.NRES
    qq = within // c.NRES

    cs, rs = src // NS, src % NS
    rp = perm[rs]
    ts, ps = rp // 128, rp % 128
    reg = (ts >= c.HT).astype(np.int64)
    row = cs * c.HR + ps * c.HT + (ts - c.HT * reg)
    win = 2 * reg + row // c.W
    gix = row - (row // c.W) * c.W

    cnt = np.zeros((nc_, c.NST, c.NRES, c.NW), np.int64)
    np.add.at(cnt, (core, st, rr, win), 1)
    cap = cnt.max(axis=0)
    slots = (cap + 127) // 128

    nb = int(slots.sum())
    TOK = nb * 128
    gidx = np.zeros((nc_, 128, TOK // 16), np.int16)
    dstv = np.full((nc_, 128, nb), -1.0, np.float32)
    wv = np.zeros((nc_, 128, nb), np.float32)

    cell_g0 = np.zeros((c.NST, c.NRES, c.NW), np.int64)
    g = 0
    for s in range(c.NST):
        for e in range(c.NRES):
            for b in range(c.NW):
                cell_g0[s, e, b] = g
                g += slots[s, e, b]

    ti = np.arange(TOK)
    rows16 = (ti % 16)[None, :] + 16 * np.arange(8)[:, None]
    cols16 = ti // 16

    key = ((core * c.NST + st) * c.NRES + rr) * c.NW + win
    order = np.lexsort((gix, key))
    ksort = key[order]
    nkey = nc_ * c.NST * c.NRES * c.NW
    bounds = np.searchsorted(ksort, np.arange(nkey + 1))
    for ci in range(nc_):
        g_lin = np.zeros(TOK, np.int16)
        d_lin = np.full(TOK, -1.0, np.float32)
        w_lin = np.zeros(TOK, np.float32)
        for s in range(c.NST):
            for e in range(c.NRES):
                for b in range(c.NW):
                    kk = ((ci * c.NST + s) * c.NRES + e) * c.NW + b
                    sel = order[bounds[kk]:bounds[kk + 1]]
                    n = len(sel)
                    t0 = int(cell_g0[s, e, b]) * 128
                    g_lin[t0:t0 + n] = gix[sel].astype(np.int16)
                    d_lin[t0:t0 + n] = qq[sel].astype(np.float32)
                    w_lin[t0:t0 + n] = w[sel]
                    ns_ = int(slots[s, e, b])
                    for q0_ in range(0, ns_, 8):
                        pos = t0 + q0_ * 128
                        if g_lin[pos] < 0:
                            g_lin[pos] = 0
        for gg in range(8):
            gidx[ci, rows16[gg], cols16] = g_lin
        dstv[ci, ti % 128, ti // 128] = d_lin
        wv[ci, ti % 128, ti // 128] = w_lin
    return gidx, dstv, wv, slots


# ----------------------------------------------------------------------
# device program
# ----------------------------------------------------------------------

def build_nc(cfg, slots):
    c = cfg
    nc = bacc.Bacc(None, target_bir_lowering=False, debug=False,
                   num_swdge_queues=NQUEUES)
    NT, NSP, C_, H = c.NT, c.NSP, c.C, c.H
    L = c.L
    NB = int(slots.sum())
    TOK = NB * 128

    def dram_in(name, shape, dt=F32):
        return nc.declare_dram_parameter(name, shape, dt, isOutput=False)

    xin_T = dram_in("xin_T", [H, NT, 128], FP16)
    d_T = dram_in("d_T", [128, NT, C_], F32)
    p_T = dram_in("p_T", [C_, NSP], FP16)
    gidx = dram_in("gidx", [128, TOK // 16], I16)
    dstv = dram_in("dstv", [128, NB], F32)
    wv = dram_in("wv", [128, NB], F32)
    lin0w = dram_in("lin0w", [H, H])
    lin0b = dram_in("lin0b", [H, 1])
    lin1w = dram_in("lin1w", [H, C_])
    lin1b = dram_in("lin1b", [C_, 1])
    wceff = dram_in("wceff", [L, H, H])
    cma45 = dram_in("cma45", [C_, C_])
    cmat45 = dram_in("cmat45", [C_, C_])
    i47 = dram_in("i47", [C_, C_])
    ident = dram_in("ident", [128, 128])
    iota = dram_in("iota", [128, c.SW], FP16)
    out_T = nc.declare_dram_parameter("out_T", [C_, NSP], F32, isOutput=True)

    # internal DRAM
    x_rep = [[nc.dram_tensor(f"x_rep{h}_{pbuf}", [c.REG, H], FP16,
                             addr_space="Shared")
              for pbuf in range(2)] for h in range(2)]
    sh = [[nc.dram_tensor(f"sh{i}{'AB'[h]}", [128, c.HT, H], FP16)
           for h in range(2)] for i in range(3)]
    # feature-major fp16 x buffers; xTh[0] holds x0 and is never overwritten
    xTh = [nc.dram_tensor(f"xTh{i}", [128, NT, H], FP16) for i in range(3)]
    cen_in = nc.dram_tensor("cen_in", [C_, H], F32)
    cen_out = nc.dram_tensor("cen_out", [C_, H], F32, addr_space="Shared")

    rg = [list(range(c.ncores))]
    CURS = [0, 1, 2, 1]
    NXTS = [1, 2, 1, 2]

    cell_g0 = np.zeros((c.NST, c.NRES, c.NW), np.int64)
    g = 0
    for s in range(c.NST):
        for e in range(c.NRES):
            for b in range(c.NW):
                cell_g0[s, e, b] = g
                g += slots[s, e, b]

    with tile.TileContext(nc) as tc:
        nc.gpsimd.load_library(library_config.mlp)
        with (
            tc.tile_pool(name="const", bufs=1) as cpool,
            tc.tile_pool(name="edge", bufs=1) as epool,
            tc.tile_pool(name="bslab", bufs=2) as bpool,
            tc.tile_pool(name="oslab", bufs=2) as opool,
            tc.tile_pool(name="gt", bufs=8) as gpool,
            tc.tile_pool(name="smat", bufs=6) as spool,
            tc.tile_pool(name="sb", bufs=4) as pool,
            tc.tile_pool(name="bank", bufs=c.CHST, space="PSUM") as bankp,
            tc.tile_pool(name="ps2", bufs=2, space="PSUM") as ps2p,
            tc.tile_pool(name="psn", bufs=1, space="PSUM") as psnp,
            tc.tile_pool(name="psA", bufs=1, space="PSUM") as psAp,
        ):
            # ---- resident constants ----
            lin0w_sb = cpool.tile([H, H], F32)
            nc.sync.dma_start(lin0w_sb[:], lin0w[:, :])
            lin0w_h = cpool.tile([H, H], FP16)
            nc.vector.tensor_copy(lin0w_h[:], lin0w_sb[:])
            lin0b_sb = cpool.tile([H, 1], F32)
            nc.sync.dma_start(lin0b_sb[:], lin0b[:, :])
            lin1w_sb = cpool.tile([H, C_], F32)
            nc.sync.dma_start(lin1w_sb[:], lin1w[:, :])
            lin1w_h = cpool.tile([H, C_], FP16)
            nc.vector.tensor_copy(lin1w_h[:], lin1w_sb[:])
            lin1b_sb = cpool.tile([C_, 1], F32)
            nc.sync.dma_start(lin1b_sb[:], lin1b[:, :])
            wceff_sb = cpool.tile([H, L * H], F32)
            for i in range(L):
                nc.sync.dma_start(wceff_sb[:, i * H:(i + 1) * H], wceff[i])
            cma_sb = cpool.tile([C_, C_], F32)
            nc.sync.dma_start(cma_sb[:], cma45[:, :])
            cmat_sb = cpool.tile([C_, C_], F32)
            nc.sync.dma_start(cmat_sb[:], cmat45[:, :])
            i47_sb = cpool.tile([C_, C_], F32)
            nc.sync.dma_start(i47_sb[:], i47[:, :])
            ident_sb = cpool.tile([128, 128], F32)
            nc.sync.dma_start(ident_sb[:], ident[:, :])
            identh_sb = cpool.tile([128, 128], FP16)
            nc.vector.tensor_copy(identh_sb[:], ident_sb[:])
            i45h_sb = cpool.tile([128, 128], FP16)
            nc.vector.tensor_scalar(i45h_sb[:], ident_sb[:], 0.45, None,
                                    mybir.AluOpType.mult)
            i10h_sb = cpool.tile([128, 128], FP16)
            nc.vector.tensor_scalar(i10h_sb[:], ident_sb[:], 0.1, None,
                                    mybir.AluOpType.mult)
            iota_sb = cpool.tile([128, c.SW], FP16)
            nc.sync.dma_start(iota_sb[:], iota[:, :])

            # ---- resident edge data ----
            gi_sb = epool.tile([128, TOK // 16], I16)
            nc.sync.dma_start(gi_sb[:], gidx[:, :])
            dv_sb = epool.tile([128, NB], F32)
            nc.sync.dma_start(dv_sb[:], dstv[:, :])
            wv_sb = epool.tile([128, NB], F32)
            nc.sync.dma_start(wv_sb[:], wv[:, :])
            d_res = epool.tile([128, NT, C_], F32)
            nc.sync.dma_start(d_res[:], d_T[:, :, :])
            cen0_sb = epool.tile([C_, H], F32)
            if ACT_MOD:
                # aux for ACT-engine S-build: t=Square((iota-d)/32),
                # S=Relu(w - 4096*w*t)
                dvn_sb = epool.tile([128, NB], F32)
                nc.vector.tensor_scalar(dvn_sb[:], dv_sb[:], -1.0 / 32, None,
                                        mybir.AluOpType.mult)
                wvn_sb = epool.tile([128, NB], F32)
                nc.vector.tensor_scalar(wvn_sb[:], wv_sb[:], -4096.0, None,
                                        mybir.AluOpType.mult)

            CHT = c.CHST * 4                       # tiles per chunk (16)

            def chunk_tiles(ch):
                t0 = ch * CHT
                return t0, min(CHT, NT - t0)

            def write_sh(dst_sh, shs, t0, ntl):
                if t0 + ntl <= c.HT:
                    nc.sync.dma_start(dst_sh[0][:, t0:t0 + ntl, :],
                                      shs[:, :ntl, :])
                elif t0 >= c.HT:
                    nc.sync.dma_start(
                        dst_sh[1][:, t0 - c.HT:t0 - c.HT + ntl, :],
                        shs[:, :ntl, :])
                else:
                    n1 = c.HT - t0
                    nc.sync.dma_start(dst_sh[0][:, t0:c.HT, :],
                                      shs[:, :n1, :])
                    nc.sync.dma_start(dst_sh[1][:, 0:ntl - n1, :],
                                      shs[:, n1:ntl, :])

            # ---- lin0 (also accumulates psA0 = d^T x0 in f32) ----
            psA = psAp.tile([C_, H], F32, tag="cen")
            for ch in range(c.NCH):
                t0, ntl = chunk_tiles(ch)
                xi = bpool.tile([128, CHT, H], FP16, tag="xTs")
                nc.sync.dma_start(xi[:, :ntl, :], xin_T[:, t0:t0 + ntl, :])
                xhs = opool.tile([128, CHT, H], FP16, tag="xhs")
                shs = opool.tile([128, CHT, H], FP16, tag="shs")
                for j in range(ntl):
                    t = t0 + j
                    ps0 = ps2p.tile([H, 128], F32, tag="b")
                    nc.tensor.matmul(ps0[:], lin0w_h[:], xi[:, j, :],
                                     start=True, stop=True)
                    nc.scalar.activation(xhs[:, j, :], ps0[:], AF.Relu,
                                         bias=lin0b_sb[:, 0:1])
                    xf = pool.tile([H, 128], F32, tag="xf")
                    nc.scalar.activation(xf[:], ps0[:], AF.Relu,
                                         bias=lin0b_sb[:, 0:1])
                    psn = psnp.tile([128, 128], F32, tag="bb")
                    nc.tensor.transpose(psn[:], xf[:], ident_sb[:])
                    nc.vector.tensor_copy(shs[:, j, :], psn[:])
                    sf = pool.tile([128, H], F32, tag="sf")
                    nc.vector.tensor_copy(sf[:], psn[:])
                    nc.tensor.matmul(psA[:], d_res[:, t, :], sf[:],
                                     start=(t == 0), stop=(t == NT - 1))
                nc.sync.dma_start(xTh[0][:, t0:t0 + ntl, :], xhs[:, :ntl, :])
                write_sh(sh[0], shs, t0, ntl)
            nc.vector.tensor_copy(cen0_sb[:], psA[:])

            # ---- layers ----
            qrr = [0]

            for li in range(L):
                cur, nxt = CURS[li], NXTS[li]
                pb = li % 2
                for h in range(2):
                    nc.gpsimd.collective_compute(
                        "AllGather", mybir.AluOpType.bypass,
                        replica_groups=rg,
                        ins=[sh[cur][h].ap().opt()],
                        outs=[x_rep[h][pb].ap().opt()],
                    )

                cenp = pool.tile([C_, H], F32, tag="cenp")
                if li == 0:
                    nc.vector.tensor_scalar(cenp[:], cen0_sb[:], 1.1, None,
                                            mybir.AluOpType.mult)
                else:
                    nc.vector.tensor_scalar(cenp[:], cen0_sb[:], 0.1, None,
                                            mybir.AluOpType.mult)
                    nc.vector.tensor_add(cenp[:], cenp[:], psA[:])
                nc.sync.dma_start(cen_in[:, :], cenp[:])
                nc.gpsimd.collective_compute(
                    "AllReduce", mybir.AluOpType.add, replica_groups=rg,
                    ins=[cen_in.ap().opt()], outs=[cen_out.ap().opt()],
                )
                cen = pool.tile([C_, H], F32, tag="cen_sb")
                nc.sync.dma_start(cen[:], cen_out[:, :])

                # - r_cls from centers (Gram trick), cma pre-scaled 0.45 -
                pst = ps2p.tile([128, C_], F32, tag="b")
                nc.tensor.transpose(pst[:, :], cen[:], ident_sb[:C_, :C_])
                cT = pool.tile([128, C_], F32, tag="cT")
                nc.vector.tensor_copy(cT[:], pst[:, :])
                psg = ps2p.tile([C_, C_], F32, tag="b")
                nc.tensor.matmul(psg[:], cT[:], cT[:], start=True, stop=True)
                gg = pool.tile([C_, C_], F32, tag="gg")
                nc.vector.tensor_copy(gg[:], psg[:])
                gd = pool.tile([C_, C_], F32, tag="gd")
                nc.vector.tensor_mul(gd[:], gg[:], i47_sb[:])
                n2 = pool.tile([C_, 1], F32, tag="n2")
                nc.vector.reduce_sum(n2[:], gd[:], AxisListType.X)
                t1 = pool.tile([C_, C_], F32, tag="t1")
                nc.vector.tensor_scalar(t1[:], gg[:], -1.0, n2[:, 0:1],
                                        mybir.AluOpType.mult,
                                        mybir.AluOpType.add)
                ps1 = ps2p.tile([C_, C_], F32, tag="b")
                nc.tensor.transpose(ps1[:], t1[:], ident_sb[:C_, :C_])
                nrm = pool.tile([C_, C_], F32, tag="nrm")
                nc.vector.tensor_add(nrm[:], t1[:], ps1[:])
                nc.vector.tensor_relu(nrm[:], nrm[:])
                nc.vector.tensor_add(nrm[:], nrm[:], i47_sb[:])
                rn = pool.tile([C_, C_], F32, tag="rn")
                nc.scalar.sqrt(rn[:], nrm[:])
                inv = pool.tile([C_, C_], F32, tag="inv")
                nc.vector.reciprocal(inv[:], rn[:])
                amat = pool.tile([C_, C_], F32, tag="amat")
                nc.vector.tensor_mul(amat[:], cma_sb[:], inv[:])
                atm = pool.tile([C_, C_], F32, tag="atm")
                nc.vector.tensor_mul(atm[:], cmat_sb[:], inv[:])
                rs = pool.tile([C_, 1], F32, tag="rs")
                nc.vector.reduce_sum(rs[:], amat[:], AxisListType.X)
                psm = ps2p.tile([C_, H], F32, tag="b")
                nc.tensor.matmul(psm[:], atm[:], cen[:], start=True, stop=True)
                rcls = pool.tile([C_, H], F32, tag="rcls")
                nc.vector.tensor_scalar(rcls[:], cen[:], rs[:, 0:1], None,
                                        mybir.AluOpType.mult)
                nc.vector.tensor_sub(rcls[:], rcls[:], psm[:])
                rclsh = pool.tile([C_, H], FP16, tag="rclsh")
                nc.vector.tensor_copy(rclsh[:], rcls[:])

                # - propagate + pass B, chunked -
                for ch in range(c.NCH):
                    t0, ntl = chunk_tiles(ch)
                    st0 = ch * c.CHST
                    nst = min(c.CHST, c.NST - st0)
                    xTs = bpool.tile([128, CHT, H], FP16, tag="xTs")
                    nc.sync.dma_start(xTs[:, :ntl, :],
                                      xTh[cur][:, t0:t0 + ntl, :])
                    x0s2 = bpool.tile([128, CHT, H], FP16, tag="x0s2")
                    nc.sync.dma_start(x0s2[:, :ntl, :],
                                      xTh[0][:, t0:t0 + ntl, :])
                    pts = bpool.tile([C_, CHT * 128], FP16, tag="pts")
                    nc.sync.dma_start(pts[:, :ntl * 128],
                                      p_T[:, t0 * 128:(t0 + ntl) * 128])

                    banks = []
                    for si in range(nst):
                        st = st0 + si
                        stw_st = min(c.STW, NSP - st * c.STW)
                        sw_st = stw_st // c.NRES
                        bank = bankp.tile([H, c.STW], F32, tag="bank")
                        banks.append(bank)
                        first = [True]
                        jlast = min(3, NT - 1 - st * 4)
                        for e in range(c.NRES):
                            bsl = bank[:, e * sw_st:(e + 1) * sw_st]
                            for wnd in range(c.NW):
                                g0 = int(cell_g0[st, e, wnd])
                                ng = int(slots[st, e, wnd])
                                if ng == 0:
                                    continue
                                rep = x_rep[wnd // 2][pb]
                                base = (wnd % 2) * c.W
                                gts = []
                                for q0 in range(0, ng, 8):
                                    q1 = min(q0 + 8, ng)
                                    ntok = (q1 - q0) * 128
                                    gt = gpool.tile([128, 8, H], FP16,
                                                    tag="g")
                                    nc.gpsimd.dma_gather(
                                        gt[:, :q1 - q0, :],
                                        rep[base:base + c.W, :],
                                        gi_sb[:, (g0 + q0) * 8:(g0 + q1) * 8],
                                        num_idxs=ntok, num_idxs_reg=ntok,
                                        elem_size=H,
                                        queue_num=qrr[0] % NQUEUES,
                                    )
                                    qrr[0] += 1
                                    gts.append((gt, q0, q1 - q0))
                                for (gt, q0, nq) in gts:
                                    for k in range(nq):
                                        gl = g0 + q0 + k
                                        S = spool.tile([128, c.SW], FP16,
                                                       tag="S")
                                        if ACT_MOD and gl % ACT_MOD == 0:
                                            St = spool.tile([128, c.SW],
                                                            FP16, tag="St")
                                            nc.scalar.activation(
                                                St[:], iota_sb[:], AF.Square,
                                                bias=dvn_sb[:, gl:gl + 1],
                                                scale=1.0 / 32)
                                            nc.scalar.activation(
                                                S[:], St[:], AF.Relu,
                                                bias=wv_sb[:, gl:gl + 1],
                                                scale=wvn_sb[:, gl:gl + 1])
                                        else:
                                            nc.vector.tensor_scalar(
                                                S[:], iota_sb[:],
                                                dv_sb[:, gl:gl + 1],
                                                wv_sb[:, gl:gl + 1],
                                                mybir.AluOpType.is_equal,
                                                mybir.AluOpType.mult)
                                        nc.tensor.matmul(
                                            bsl, gt[:, k, :], S[:, :sw_st],
                                            start=first[0], stop=False,
                                            skip_group_check=True)
                                        first[0] = False
                        for j in range(4):
                            t = st * 4 + j
                            if t >= NT:
                                break
                            jj = t - t0
                            sl = bank[:, j * 128:(j + 1) * 128]
                            nc.tensor.matmul(sl, rclsh[:],
                                             pts[:, jj * 128:(jj + 1) * 128],
                                             start=first[0], stop=False,
                                             skip_group_check=True)
                            nc.tensor.matmul(sl, i45h_sb[:], xTs[:, jj, :],
                                             start=False, stop=False,
                                             skip_group_check=True)
                            nc.tensor.matmul(sl, i10h_sb[:], x0s2[:, jj, :],
                                             start=False, stop=(j == jlast),
                                             skip_group_check=True)
                            first[0] = False

                    # pass B compute per tile (also accumulates next
                    # layer's centers from f32 data)
                    if ch == 0 and li < L - 1:
                        psA = psAp.tile([C_, H], F32, tag="cen")
                    xhs = opool.tile([128, CHT, H], FP16, tag="xhs")
                    shs = opool.tile([128, CHT, H], FP16, tag="shs")
                    for si in range(nst):
                        st = st0 + si
                        for j in range(4):
                            t = st * 4 + j
                            if t >= NT:
                                break
                            jj = t - t0
                            u = pool.tile([H, 128], F32, tag="u")
                            nc.vector.tensor_copy(
                                u[:], banks[si][:, j * 128:(j + 1) * 128])
                            ps2 = ps2p.tile([H, 128], F32, tag="b")
                            nc.tensor.matmul(ps2[:],
                                             wceff_sb[:, li * H:(li + 1) * H],
                                             u[:], start=True, stop=True)
                            nc.scalar.activation(xhs[:, jj, :], ps2[:],
                                                 AF.Relu)
                            xf = pool.tile([H, 128], F32, tag="xf")
                            nc.scalar.activation(xf[:], ps2[:], AF.Relu)
                            psn = psnp.tile([128, 128], F32, tag="bb")
                            nc.tensor.transpose(psn[:], xf[:], ident_sb[:])
                            nc.vector.tensor_copy(shs[:, jj, :], psn[:])
                            if li < L - 1:
                                sf = pool.tile([128, H], F32, tag="sf")
                                nc.vector.tensor_copy(sf[:], psn[:])
                                nc.tensor.matmul(psA[:], d_res[:, t, :],
                                                 sf[:], start=(t == 0),
                                                 stop=(t == NT - 1))
                    nc.sync.dma_start(xTh[nxt][:, t0:t0 + ntl, :],
                                      xhs[:, :ntl, :])
                    write_sh(sh[nxt], shs, t0, ntl)

            # ---- lin1 ----
            fin = NXTS[L - 1]
            for ch in range(c.NCH):
                t0, ntl = chunk_tiles(ch)
                xi = bpool.tile([128, CHT, H], FP16, tag="xTs")
                nc.sync.dma_start(xi[:, :ntl, :], xTh[fin][:, t0:t0 + ntl, :])
                ots = opool.tile([C_, CHT * 128], F32, tag="ots", bufs=1)
                for j in range(ntl):
                    psf = ps2p.tile([C_, 128], F32, tag="b")
                    nc.tensor.matmul(psf[:], lin1w_h[:], xi[:, j, :],
                                     start=True, stop=True)
                    nc.vector.tensor_scalar(ots[:, j * 128:(j + 1) * 128],
                                            psf[:], lin1b_sb[:, 0:1], None,
                                            mybir.AluOpType.add)
                nc.sync.dma_start(out_T[:, t0 * 128:(t0 + ntl) * 128],
                                  ots[:, :ntl * 128])

    nc.compile()
    return nc


def _load_sh_slab(nc, c, shp, dest, t0, ntl):
    """Load node-major tiles [t0, t0+ntl) from half-split sh into dest."""
    if t0 + ntl <= c.HT:
        nc.sync.dma_start(dest[:, :ntl, :], shp[0][:, t0:t0 + ntl, :])
    elif t0 >= c.HT:
        nc.sync.dma_start(dest[:, :ntl, :],
                          shp[1][:, t0 - c.HT:t0 - c.HT + ntl, :])
    else:
        n1 = c.HT - t0
        nc.sync.dma_start(dest[:, :n1, :], shp[0][:, t0:c.HT, :])
        nc.sync.dma_start(dest[:, n1:ntl, :], shp[1][:, 0:ntl - n1, :])


# ----------------------------------------------------------------------
# host wrapper
# ----------------------------------------------------------------------

def _prep_inputs(cfg, inputs):
    c = cfg
    x = np.asarray(inputs["x"], np.float32)
    label = np.asarray(inputs["label"], np.int64)
    p = np.asarray(inputs["p"], np.float32)
    cm = np.asarray(inputs["cm"], np.float32)
    lin0_w = np.asarray(inputs["lin0_w"], np.float32)
    lin0_b = np.asarray(inputs["lin0_b"], np.float32)
    lin1_w = np.asarray(inputs["lin1_w"], np.float32)
    lin1_b = np.asarray(inputs["lin1_b"], np.float32)
    conv_w = np.asarray(inputs["conv_w"], np.float32)

    gidx, dstv, wv, slots = _prep_edges(cfg, inputs["edge_index"],
                                        inputs["edge_weight"])

    cnt = np.bincount(label, minlength=c.C).astype(np.float32)
    cnt = np.maximum(cnt, 1.0)
    cma = cm[:, 0, :] * (c.rsl * (1.0 - c.alpha))      # 0.45 fold
    i47 = np.eye(c.C, dtype=np.float32)
    ident = np.eye(128, dtype=np.float32)
    iota = np.tile(np.arange(c.SW, dtype=np.float16)[None, :], (128, 1))
    wceff = np.zeros((c.L, c.H, c.H), np.float32)
    for i in range(c.L):
        beta = float(np.log(c.theta / (i + 1) + 1.0))
        wceff[i] = (1.0 - beta) * np.eye(c.H, dtype=np.float32) \
            + beta * conv_w[i]

    perm = _node_perm(c)
    pidx = perm[np.arange(c.NS)]
    in_maps = []
    for ci in range(c.ncores):
        r0 = ci * c.NS
        xs = np.zeros((c.NSP, c.H), np.float32)
        xs[pidx] = x[r0:r0 + c.NS]
        lab = label[r0:r0 + c.NS]
        d_t = np.zeros((c.NSP, c.C), np.float32)
        d_t[pidx, lab] = 1.0 / cnt[lab]
        p_pad = np.zeros((c.NSP, c.C), np.float32)
        p_pad[pidx] = p[r0:r0 + c.NS]
        in_maps.append({
            "xin_T": np.ascontiguousarray(xs.T).reshape(
                c.H, c.NT, 128).astype(np.float16),
            "d_T": np.ascontiguousarray(
                d_t.reshape(c.NT, 128, c.C).transpose(1, 0, 2)),
            "p_T": np.ascontiguousarray(p_pad.T).astype(np.float16),
            "gidx": gidx[ci], "dstv": dstv[ci], "wv": wv[ci],
            "lin0w": lin0_w, "lin0b": lin0_b.reshape(-1, 1),
            "lin1w": lin1_w, "lin1b": lin1_b.reshape(-1, 1),
            "wceff": wceff, "cma45": cma,
            "cmat45": np.ascontiguousarray(cma.T),
            "i47": i47, "ident": ident, "iota": iota,
        })
    return in_maps, slots


_BUILT = {}


def kernel(**inputs):
    cfg = DEF
    in_maps, slots = _prep_inputs(cfg, inputs)
    key = "default"
    if key not in _BUILT:
        _BUILT[key] = build_nc(cfg, slots)
    nc = _BUILT[key]
    res = bass_utils.run_bass_kernel_spmd(nc, in_maps,
                                          core_ids=list(range(cfg.ncores)))
    pidx = _node_perm(cfg)[np.arange(cfg.NS)]
    outs = [res.results[ci]["out_T"].T[pidx] for ci in range(cfg.ncores)]
    return np.ascontiguousarray(np.concatenate(outs, 0).astype(np.float32))


# revision 15
# speedup vs baseline: 3.1414x; 1.0342x over previous
"""GCN2 (nn_GCN2_42331197669873) Bass kernel for 8 TRN2 NeuronCores.

Strategy: graph/data parallel, nodes sharded row-wise (12544 padded rows
per core).  The sparse propagate is a one-hot-matmul segment sum: edge
source rows are gathered (SWDGE dma_gather spread over 4 queues = 4 Q7
core pairs, ~3.1 ns/token vs 8.7 single-queue) as fp16 rows into SBUF
token groups of 128; each group is multiplied on the tensor engine by an
on-the-fly selection matrix S[tok, dst] = w * (dst_off(tok)==col),
accumulating into a per-supertile PSUM bank [H, 512].  No scatter-add,
no HBM aggregation buffers, f32 accumulation.  The dense combine
(class-center term, 0.45x + 0.1x0, beta-folded conv) continues in the
same PSUM bank via matmul injections, so there is one PSUM round trip
per tile per layer.

All x-valued tensors are fp16: the class centers are nearly identical
(||c|| ~ 28x the pairwise distances), so the normalized class-difference
term amplifies center errors ~8x and bf16 node features are not accurate
enough.  fp16 also keeps the S-build is_equal compare exact (integers
<= 2048).

Node features are replicated per layer with two half AllGathers
(double-buffered x_rep so the next layer's AG overlaps this layer's
tail).  Weights/class tensors are tiny and resident.

kernel(**inputs) takes the FULL unsharded inputs and returns the FULL
[100000, 47] output; sharding + preprocessing happens on host inside.
"""

import numpy as np
import ml_dtypes

from concourse import bass, bacc, tile, mybir, bass_utils
from concourse import library_config
from concourse.mybir import AxisListType
import concourse.tile_sem_assignment as _tsa
from concourse import bass_isa as _bisa

NQUEUES = 4
ACT_MOD = 3          # every ACT_MOD-th S-build goes to the ACT engine (0=off)

# Tile round-robins Pool-engine DMAs over all DMASW sem lanes ignoring
# queue_num; mixing SWDGE queues on one lane breaks its in-order-completion
# assumption (sim: "sem locked to SWDGE queue"). Segregate lanes by queue:
# queue q -> lanes [2q, 2q+1].
_orig_assign_tick = _tsa.TileClockTick._assign_tick


def _assign_tick_qsplit(self, inst):
    if (isinstance(inst, _tsa.DMAInst)
            and inst.engine == mybir.EngineType.Pool
            and not isinstance(inst, _bisa.UserSyncedRemoteDMADescs)
            and self.swdge_sem_count >= NQUEUES * 2):
        qn = getattr(inst, "queue_num", 0) or 0
        lanes = self.swdge_sem_count // NQUEUES
        if not hasattr(self, "_qrr"):
            self._qrr = {}
        r = self._qrr.get(qn, 0)
        self._qrr[qn] = r + 1
        self.next_sw_dma_idx = (qn % NQUEUES) * lanes + r % lanes
    return _orig_assign_tick(self, inst)


_tsa.TileClockTick._assign_tick = _assign_tick_qsplit

F32 = mybir.dt.float32
BF16 = mybir.dt.bfloat16
FP16 = mybir.dt.float16
I16 = mybir.dt.int16
AF = mybir.ActivationFunctionType


class Cfg:
    def __init__(self):
        self.N, self.E, self.C, self.H = 100000, 800000, 47, 128
        self.ncores = 8
        self.NS = self.N // self.ncores      # real nodes per core
        self.NT = 98                         # 128-row tiles per core
        self.NSP = self.NT * 128             # padded nodes per core
        self.HT = 49                         # tiles per half
        self.HR = self.HT * 128              # rows per half shard (6272)
        self.REG = self.ncores * self.HR     # x_rep region rows (50176)
        self.W = self.REG // 2               # gather window rows (25088)
        self.NW = 4                          # windows (2 per region)
        self.STW = 512                       # supertile width (psum bank)
        self.NST = (self.NSP + 511) // 512   # supertiles per core (25)
        self.NRES = 2                        # node interleave classes
        self.SW = self.STW // self.NRES      # S matrix width (256)
        self.CHST = 4                        # supertiles per chunk
        self.NCH = (self.NST + 3) // 4       # chunks (7)
        self.L = 4
        self.alpha, self.theta, self.rsl = 0.1, 0.5, 0.5


DEF = Cfg()


# ----------------------------------------------------------------------
# host-side edge preprocessing
# ----------------------------------------------------------------------

def _node_perm(cfg):
    """Physical position of each padded local node index (within-supertile
    interleave by residue class, so S matrices are STW/NRES wide)."""
    c = cfg
    r = np.arange(c.NSP)
    st = r // c.STW
    stw = np.minimum(c.STW, c.NSP - st * c.STW)
    sw = stw // c.NRES
    w = r - st * c.STW
    return st * c.STW + (w % c.NRES) * sw + w // c.NRES


def _prep_edges(cfg, edge_index, edge_weight):
    """Token layout: cells (supertile st, residue rr, window w); slots =
    128-multiple max-over-cores capacity; tokens sorted by gather idx,
    trailing pads use idx -1 (trimmed by the Q7 per core).

    Returns gidx [nc,128,COLS] i16, dstv/wv [nc,128,NB] f32,
    slots [NST][NRES][NW].
    """
    c = cfg
    perm = _node_perm(c)
    src = np.asarray(edge_index[0], np.int64)
    dst = np.asarray(edge_index[1], np.int64)
    w = np.asarray(edge_weight, np.float32) * (1.0 - c.rsl) * (1.0 - c.alpha)
    nc_, NS = c.ncores, c.NS

    core = dst // NS
    r = dst - core * NS
    st = r // c.STW
    stw = np.minimum(c.STW, c.NSP - st * c.STW)
    sw = stw // c.NRES
    within = r - st * c.STW
    rr = within % c.NRES
    qq = within // c.NRES

    cs, rs = src // NS, src % NS
    rp = perm[rs]
    ts, ps = rp // 128, rp % 128
    reg = (ts >= c.HT).astype(np.int64)
    row = cs * c.HR + ps * c.HT + (ts - c.HT * reg)
    win = 2 * reg + row // c.W
    gix = row - (row // c.W) * c.W

    cnt = np.zeros((nc_, c.NST, c.NRES, c.NW), np.int64)
    np.add.at(cnt, (core, st, rr, win), 1)
    cap = cnt.max(axis=0)
    slots = (cap + 127) // 128

    nb = int(slots.sum())
    TOK = nb * 128
    gidx = np.zeros((nc_, 128, TOK // 16), np.int16)
    dstv = np.full((nc_, 128, nb), -1.0, np.float32)
    wv = np.zeros((nc_, 128, nb), np.float32)

    cell_g0 = np.zeros((c.NST, c.NRES, c.NW), np.int64)
    g = 0
    for s in range(c.NST):
        for e in range(c.NRES):
            for b in range(c.NW):
                cell_g0[s, e, b] = g
                g += slots[s, e, b]

    ti = np.arange(TOK)
    rows16 = (ti % 16)[None, :] + 16 * np.arange(8)[:, None]
    cols16 = ti // 16

    key = ((core * c.NST + st) * c.NRES + rr) * c.NW + win
    order = np.lexsort((gix, key))
    ksort = key[order]
    nkey = nc_ * c.NST * c.NRES * c.NW
    bounds = np.searchsorted(ksort, np.arange(nkey + 1))
    for ci in range(nc_):
        g_lin = np.zeros(TOK, np.int16)
        d_lin = np.full(TOK, -1.0, np.float32)
        w_lin = np.zeros(TOK, np.float32)
        for s in range(c.NST):
            for e in range(c.NRES):
                for b in range(c.NW):
                    kk = ((ci * c.NST + s) * c.NRES + e) * c.NW + b
                    sel = order[bounds[kk]:bounds[kk + 1]]
                    n = len(sel)
                    t0 = int(cell_g0[s, e, b]) * 128
                    g_lin[t0:t0 + n] = gix[sel].astype(np.int16)
                    d_lin[t0:t0 + n] = qq[sel].astype(np.float32)
                    w_lin[t0:t0 + n] = w[sel]
                    ns_ = int(slots[s, e, b])
                    for q0_ in range(0, ns_, 8):
                        pos = t0 + q0_ * 128
                        if g_lin[pos] < 0:
                            g_lin[pos] = 0
        for gg in range(8):
            gidx[ci, rows16[gg], cols16] = g_lin
        dstv[ci, ti % 128, ti // 128] = d_lin
        wv[ci, ti % 128, ti // 128] = w_lin
    return gidx, dstv, wv, slots


# ----------------------------------------------------------------------
# device program
# ----------------------------------------------------------------------

def build_nc(cfg, slots):
    c = cfg
    nc = bacc.Bacc(None, target_bir_lowering=False, debug=False,
                   num_swdge_queues=NQUEUES)
    NT, NSP, C_, H = c.NT, c.NSP, c.C, c.H
    L = c.L
    NB = int(slots.sum())
    TOK = NB * 128

    def dram_in(name, shape, dt=F32):
        return nc.declare_dram_parameter(name, shape, dt, isOutput=False)

    xin_T = dram_in("xin_T", [H, NT, 128], FP16)
    d_T = dram_in("d_T", [128, NT, C_], F32)
    p_T = dram_in("p_T", [C_, NSP], FP16)
    gidx = dram_in("gidx", [128, TOK // 16], I16)
    dstv = dram_in("dstv", [128, NB], F32)
    wv = dram_in("wv", [128, NB], F32)
    lin0w = dram_in("lin0w", [H, H])
    lin0b = dram_in("lin0b", [H, 1])
    lin1w = dram_in("lin1w", [H, C_])
    lin1b = dram_in("lin1b", [C_, 1])
    wceff = dram_in("wceff", [L, H, H])
    cma45 = dram_in("cma45", [C_, C_])
    cmat45 = dram_in("cmat45", [C_, C_])
    i47 = dram_in("i47", [C_, C_])
    ident = dram_in("ident", [128, 128])
    iota = dram_in("iota", [128, c.SW], FP16)
    out_T = nc.declare_dram_parameter("out_T", [C_, NSP], F32, isOutput=True)

    # internal DRAM
    x_rep = [[nc.dram_tensor(f"x_rep{h}_{pbuf}", [c.REG, H], FP16,
                             addr_space="Shared")
              for pbuf in range(2)] for h in range(2)]
    sh = [[nc.dram_tensor(f"sh{i}{'AB'[h]}", [128, c.HT, H], FP16)
           for h in range(2)] for i in range(3)]
    # feature-major fp16 x buffers; xTh[0] holds x0 and is never overwritten
    xTh = [nc.dram_tensor(f"xTh{i}", [128, NT, H], FP16) for i in range(3)]
    cen_in = nc.dram_tensor("cen_in", [C_, H], F32)
    cen_out = nc.dram_tensor("cen_out", [C_, H], F32, addr_space="Shared")

    rg = [list(range(c.ncores))]
    CURS = [0, 1, 2, 1]
    NXTS = [1, 2, 1, 2]

    cell_g0 = np.zeros((c.NST, c.NRES, c.NW), np.int64)
    g = 0
    for s in range(c.NST):
        for e in range(c.NRES):
            for b in range(c.NW):
                cell_g0[s, e, b] = g
                g += slots[s, e, b]

    with tile.TileContext(nc) as tc:
        nc.gpsimd.load_library(library_config.mlp)
        with (
            tc.tile_pool(name="const", bufs=1) as cpool,
            tc.tile_pool(name="edge", bufs=1) as epool,
            tc.tile_pool(name="bslab", bufs=2) as bpool,
            tc.tile_pool(name="oslab", bufs=2) as opool,
            tc.tile_pool(name="gt", bufs=8) as gpool,
            tc.tile_pool(name="smat", bufs=6) as spool,
            tc.tile_pool(name="sb", bufs=4) as pool,
            tc.tile_pool(name="bank", bufs=c.CHST, space="PSUM") as bankp,
            tc.tile_pool(name="ps2", bufs=2, space="PSUM") as ps2p,
            tc.tile_pool(name="psn", bufs=1, space="PSUM") as psnp,
            tc.tile_pool(name="psA", bufs=1, space="PSUM") as psAp,
        ):
            # ---- resident constants ----
            lin0w_sb = cpool.tile([H, H], F32)
            nc.sync.dma_start(lin0w_sb[:], lin0w[:, :])
            lin0w_h = cpool.tile([H, H], FP16)
            nc.vector.tensor_copy(lin0w_h[:], lin0w_sb[:])
            lin0b_sb = cpool.tile([H, 1], F32)
            nc.sync.dma_start(lin0b_sb[:], lin0b[:, :])
            lin1w_sb = cpool.tile([H, C_], F32)
            nc.sync.dma_start(lin1w_sb[:], lin1w[:, :])
            lin1w_h = cpool.tile([H, C_], FP16)
            nc.vector.tensor_copy(lin1w_h[:], lin1w_sb[:])
            lin1b_sb = cpool.tile([C_, 1], F32)
            nc.sync.dma_start(lin1b_sb[:], lin1b[:, :])
            wceff_sb = cpool.tile([H, L * H], F32)
            for i in range(L):
                nc.sync.dma_start(wceff_sb[:, i * H:(i + 1) * H], wceff[i])
            cma_sb = cpool.tile([C_, C_], F32)
            nc.sync.dma_start(cma_sb[:], cma45[:, :])
            cmat_sb = cpool.tile([C_, C_], F32)
            nc.sync.dma_start(cmat_sb[:], cmat45[:, :])
            i47_sb = cpool.tile([C_, C_], F32)
            nc.sync.dma_start(i47_sb[:], i47[:, :])
            ident_sb = cpool.tile([128, 128], F32)
            nc.sync.dma_start(ident_sb[:], ident[:, :])
            identh_sb = cpool.tile([128, 128], FP16)
            nc.vector.tensor_copy(identh_sb[:], ident_sb[:])
            i45h_sb = cpool.tile([128, 128], FP16)
            nc.vector.tensor_scalar(i45h_sb[:], ident_sb[:], 0.45, None,
                                    mybir.AluOpType.mult)
            i10h_sb = cpool.tile([128, 128], FP16)
            nc.vector.tensor_scalar(i10h_sb[:], ident_sb[:], 0.1, None,
                                    mybir.AluOpType.mult)
            iota_sb = cpool.tile([128, c.SW], FP16)
            nc.sync.dma_start(iota_sb[:], iota[:, :])

            # ---- resident edge data ----
            gi_sb = epool.tile([128, TOK // 16], I16)
            nc.sync.dma_start(gi_sb[:], gidx[:, :])
            dv_sb = epool.tile([128, NB], F32)
            nc.sync.dma_start(dv_sb[:], dstv[:, :])
            wv_sb = epool.tile([128, NB], F32)
            nc.sync.dma_start(wv_sb[:], wv[:, :])
            d_res = epool.tile([128, NT, C_], F32)
            nc.sync.dma_start(d_res[:], d_T[:, :, :])
            cen0_sb = epool.tile([C_, H], F32)
            if ACT_MOD:
                # aux for ACT-engine S-build: t=Square((iota-d)/32),
                # S=Relu(w - 4096*w*t)
                dvn_sb = epool.tile([128, NB], F32)
                nc.vector.tensor_scalar(dvn_sb[:], dv_sb[:], -1.0 / 32, None,
                                        mybir.AluOpType.mult)
                wvn_sb = epool.tile([128, NB], F32)
                nc.vector.tensor_scalar(wvn_sb[:], wv_sb[:], -4096.0, None,
                                        mybir.AluOpType.mult)

            CHT = c.CHST * 4                       # tiles per chunk (16)

            def chunk_tiles(ch):
                t0 = ch * CHT
                return t0, min(CHT, NT - t0)

            def write_sh(dst_sh, shs, t0, ntl):
                if t0 + ntl <= c.HT:
                    nc.sync.dma_start(dst_sh[0][:, t0:t0 + ntl, :],
                                      shs[:, :ntl, :])
                elif t0 >= c.HT:
                    nc.sync.dma_start(
                        dst_sh[1][:, t0 - c.HT:t0 - c.HT + ntl, :],
                        shs[:, :ntl, :])
                else:
                    n1 = c.HT - t0
                    nc.sync.dma_start(dst_sh[0][:, t0:c.HT, :],
                                      shs[:, :n1, :])
                    nc.sync.dma_start(dst_sh[1][:, 0:ntl - n1, :],
                                      shs[:, n1:ntl, :])

            # ---- lin0 (also accumulates psA0 = d^T x0 in f32) ----
            psA = psAp.tile([C_, H], F32, tag="cen")
            for ch in range(c.NCH):
                t0, ntl = chunk_tiles(ch)
                xi = bpool.tile([128, CHT, H], FP16, tag="xTs")
                nc.sync.dma_start(xi[:, :ntl, :], xin_T[:, t0:t0 + ntl, :])
                xhs = opool.tile([128, CHT, H], FP16, tag="xhs")
                shs = opool.tile([128, CHT, H], FP16, tag="shs")
                for j in range(ntl):
                    t = t0 + j
                    ps0 = ps2p.tile([H, 128], F32, tag="b")
                    nc.tensor.matmul(ps0[:], lin0w_h[:], xi[:, j, :],
                                     start=True, stop=True)
                    nc.scalar.activation(xhs[:, j, :], ps0[:], AF.Relu,
                                         bias=lin0b_sb[:, 0:1])
                    xf = pool.tile([H, 128], F32, tag="xf")
                    nc.scalar.activation(xf[:], ps0[:], AF.Relu,
                                         bias=lin0b_sb[:, 0:1])
                    psn = psnp.tile([128, 128], F32, tag="bb")
                    nc.tensor.transpose(psn[:], xf[:], ident_sb[:])
                    nc.vector.tensor_copy(shs[:, j, :], psn[:])
                    sf = pool.tile([128, H], F32, tag="sf")
                    nc.vector.tensor_copy(sf[:], psn[:])
                    nc.tensor.matmul(psA[:], d_res[:, t, :], sf[:],
                                     start=(t == 0), stop=(t == NT - 1))
                nc.sync.dma_start(xTh[0][:, t0:t0 + ntl, :], xhs[:, :ntl, :])
                write_sh(sh[0], shs, t0, ntl)
            nc.vector.tensor_copy(cen0_sb[:], psA[:])

            # ---- layers ----
            qrr = [0]

            for li in range(L):
                cur, nxt = CURS[li], NXTS[li]
                pb = li % 2
                for h in range(2):
                    nc.gpsimd.collective_compute(
                        "AllGather", mybir.AluOpType.bypass,
                        replica_groups=rg,
                        ins=[sh[cur][h].ap().opt()],
                        outs=[x_rep[h][pb].ap().opt()],
                    )

                cenp = pool.tile([C_, H], F32, tag="cenp")
                if li == 0:
                    nc.vector.tensor_scalar(cenp[:], cen0_sb[:], 1.1, None,
                                            mybir.AluOpType.mult)
                else:
                    nc.vector.tensor_scalar(cenp[:], cen0_sb[:], 0.1, None,
                                            mybir.AluOpType.mult)
                    nc.vector.tensor_add(cenp[:], cenp[:], psA[:])
                nc.sync.dma_start(cen_in[:, :], cenp[:])
                nc.gpsimd.collective_compute(
                    "AllReduce", mybir.AluOpType.add, replica_groups=rg,
                    ins=[cen_in.ap().opt()], outs=[cen_out.ap().opt()],
                )
                cen = pool.tile([C_, H], F32, tag="cen_sb")
                nc.sync.dma_start(cen[:], cen_out[:, :])

                # - r_cls from centers (Gram trick), cma pre-scaled 0.45 -
                pst = ps2p.tile([128, C_], F32, tag="b")
                nc.tensor.transpose(pst[:, :], cen[:], ident_sb[:C_, :C_])
                cT = pool.tile([128, C_], F32, tag="cT")
                nc.vector.tensor_copy(cT[:], pst[:, :])
                psg = ps2p.tile([C_, C_], F32, tag="b")
                nc.tensor.matmul(psg[:], cT[:], cT[:], start=True, stop=True)
                gg = pool.tile([C_, C_], F32, tag="gg")
                nc.vector.tensor_copy(gg[:], psg[:])
                gd = pool.tile([C_, C_], F32, tag="gd")
                nc.vector.tensor_mul(gd[:], gg[:], i47_sb[:])
                n2 = pool.tile([C_, 1], F32, tag="n2")
                nc.vector.reduce_sum(n2[:], gd[:], AxisListType.X)
                t1 = pool.tile([C_, C_], F32, tag="t1")
                nc.vector.tensor_scalar(t1[:], gg[:], -1.0, n2[:, 0:1],
                                        mybir.AluOpType.mult,
                                        mybir.AluOpType.add)
                ps1 = ps2p.tile([C_, C_], F32, tag="b")
                nc.tensor.transpose(ps1[:], t1[:], ident_sb[:C_, :C_])
                nrm = pool.tile([C_, C_], F32, tag="nrm")
                nc.vector.tensor_add(nrm[:], t1[:], ps1[:])
                nc.vector.tensor_relu(nrm[:], nrm[:])
                nc.vector.tensor_add(nrm[:], nrm[:], i47_sb[:])
                rn = pool.tile([C_, C_], F32, tag="rn")
                nc.scalar.sqrt(rn[:], nrm[:])
                inv = pool.tile([C_, C_], F32, tag="inv")
                nc.vector.reciprocal(inv[:], rn[:])
                amat = pool.tile([C_, C_], F32, tag="amat")
                nc.vector.tensor_mul(amat[:], cma_sb[:], inv[:])
                atm = pool.tile([C_, C_], F32, tag="atm")
                nc.vector.tensor_mul(atm[:], cmat_sb[:], inv[:])
                rs = pool.tile([C_, 1], F32, tag="rs")
                nc.vector.reduce_sum(rs[:], amat[:], AxisListType.X)
                psm = ps2p.tile([C_, H], F32, tag="b")
                nc.tensor.matmul(psm[:], atm[:], cen[:], start=True, stop=True)
                rcls = pool.tile([C_, H], F32, tag="rcls")
                nc.vector.tensor_scalar(rcls[:], cen[:], rs[:, 0:1], None,
                                        mybir.AluOpType.mult)
                nc.vector.tensor_sub(rcls[:], rcls[:], psm[:])
                rclsh = pool.tile([C_, H], FP16, tag="rclsh")
                nc.vector.tensor_copy(rclsh[:], rcls[:])

                # - propagate + pass B, chunked -
                for ch in range(c.NCH):
                    t0, ntl = chunk_tiles(ch)
                    st0 = ch * c.CHST
                    nst = min(c.CHST, c.NST - st0)
                    xTs = bpool.tile([128, CHT, H], FP16, tag="xTs")
                    nc.sync.dma_start(xTs[:, :ntl, :],
                                      xTh[cur][:, t0:t0 + ntl, :])
                    x0s2 = bpool.tile([128, CHT, H], FP16, tag="x0s2")
                    nc.sync.dma_start(x0s2[:, :ntl, :],
                                      xTh[0][:, t0:t0 + ntl, :])
                    pts = bpool.tile([C_, CHT * 128], FP16, tag="pts")
                    nc.sync.dma_start(pts[:, :ntl * 128],
                                      p_T[:, t0 * 128:(t0 + ntl) * 128])

                    banks = []
                    for si in range(nst):
                        st = st0 + si
                        stw_st = min(c.STW, NSP - st * c.STW)
                        sw_st = stw_st // c.NRES
                        bank = bankp.tile([H, c.STW], F32, tag="bank")
                        banks.append(bank)
                        first = [True]
                        jlast = min(3, NT - 1 - st * 4)
                        for e in range(c.NRES):
                            bsl = bank[:, e * sw_st:(e + 1) * sw_st]
                            for wnd in range(c.NW):
                                g0 = int(cell_g0[st, e, wnd])
                                ng = int(slots[st, e, wnd])
                                if ng == 0:
                                    continue
                                rep = x_rep[wnd // 2][pb]
                                base = (wnd % 2) * c.W
                                gts = []
                                for q0 in range(0, ng, 8):
                                    q1 = min(q0 + 8, ng)
                                    ntok = (q1 - q0) * 128
                                    gt = gpool.tile([128, 8, H], FP16,
                                                    tag="g")
                                    nc.gpsimd.dma_gather(
                                        gt[:, :q1 - q0, :],
                                        rep[base:base + c.W, :],
                                        gi_sb[:, (g0 + q0) * 8:(g0 + q1) * 8],
                                        num_idxs=ntok, num_idxs_reg=ntok,
                                        elem_size=H,
                                        queue_num=qrr[0] % NQUEUES,
                                    )
                                    qrr[0] += 1
                                    gts.append((gt, q0, q1 - q0))
                                for (gt, q0, nq) in gts:
                                    for k in range(nq):
                                        gl = g0 + q0 + k
                                        S = spool.tile([128, c.SW], FP16,
                                                       tag="S")
                                        if ACT_MOD and gl % ACT_MOD == 0:
                                            St = spool.tile([128, c.SW],
                                                            FP16, tag="St")
                                            nc.scalar.activation(
                                                St[:], iota_sb[:], AF.Square,
                                                bias=dvn_sb[:, gl:gl + 1],
                                                scale=1.0 / 32)
                                            nc.scalar.activation(
                                                S[:], St[:], AF.Relu,
                                                bias=wv_sb[:, gl:gl + 1],
                                                scale=wvn_sb[:, gl:gl + 1])
                                        else:
                                            nc.vector.tensor_scalar(
                                                S[:], iota_sb[:],
                                                dv_sb[:, gl:gl + 1],
                                                wv_sb[:, gl:gl + 1],
                                                mybir.AluOpType.is_equal,
                                                mybir.AluOpType.mult)
                                        nc.tensor.matmul(
                                            bsl, gt[:, k, :], S[:, :sw_st],
                                            start=first[0], stop=False,
                                            skip_group_check=True)
                                        first[0] = False
                        for j in range(4):
                            t = st * 4 + j
                            if t >= NT:
                                break
                            jj = t - t0
                            sl = bank[:, j * 128:(j + 1) * 128]
                            nc.tensor.matmul(sl, rclsh[:],
                                             pts[:, jj * 128:(jj + 1) * 128],
                                             start=first[0], stop=False,
                                             skip_group_check=True)
                            nc.tensor.matmul(sl, i45h_sb[:], xTs[:, jj, :],
                                             start=False, stop=False,
                                             skip_group_check=True)
                            nc.tensor.matmul(sl, i10h_sb[:], x0s2[:, jj, :],
                                             start=False, stop=(j == jlast),
                                             skip_group_check=True)
                            first[0] = False

                    # pass B compute per tile (also accumulates next
                    # layer's centers from f32 data)
                    if ch == 0 and li < L - 1:
                        psA = psAp.tile([C_, H], F32, tag="cen")
                    xhs = opool.tile([128, CHT, H], FP16, tag="xhs")
                    shs = opool.tile([128, CHT, H], FP16, tag="shs")
                    for si in range(nst):
                        st = st0 + si
                        for j in range(4):
                            t = st * 4 + j
                            if t >= NT:
                                break
                            jj = t - t0
                            u = pool.tile([H, 128], F32, tag="u")
                            nc.vector.tensor_copy(
                                u[:], banks[si][:, j * 128:(j + 1) * 128])
                            ps2 = ps2p.tile([H, 128], F32, tag="b")
                            nc.tensor.matmul(ps2[:],
                                             wceff_sb[:, li * H:(li + 1) * H],
                                             u[:], start=True, stop=True)
                            nc.scalar.activation(xhs[:, jj, :], ps2[:],
                                                 AF.Relu)
                            xf = pool.tile([H, 128], F32, tag="xf")
                            nc.scalar.activation(xf[:], ps2[:], AF.Relu)
                            psn = psnp.tile([128, 128], F32, tag="bb")
                            nc.tensor.transpose(psn[:], xf[:], ident_sb[:])
                            nc.vector.tensor_copy(shs[:, jj, :], psn[:])
                            if li < L - 1:
                                sf = pool.tile([128, H], F32, tag="sf")
                                nc.vector.tensor_copy(sf[:], psn[:])
                                nc.tensor.matmul(psA[:], d_res[:, t, :],
                                                 sf[:], start=(t == 0),
                                                 stop=(t == NT - 1))
                    nc.sync.dma_start(xTh[nxt][:, t0:t0 + ntl, :],
                                      xhs[:, :ntl, :])
                    write_sh(sh[nxt], shs, t0, ntl)

            # ---- lin1 ----
            fin = NXTS[L - 1]
            for ch in range(c.NCH):
                t0, ntl = chunk_tiles(ch)
                xi = bpool.tile([128, CHT, H], FP16, tag="xTs")
                nc.sync.dma_start(xi[:, :ntl, :], xTh[fin][:, t0:t0 + ntl, :])
                ots = opool.tile([C_, CHT * 128], F32, tag="ots", bufs=1)
                for j in range(ntl):
                    psf = ps2p.tile([C_, 128], F32, tag="b")
                    nc.tensor.matmul(psf[:], lin1w_h[:], xi[:, j, :],
                                     start=True, stop=True)
                    nc.vector.tensor_scalar(ots[:, j * 128:(j + 1) * 128],
                                            psf[:], lin1b_sb[:, 0:1], None,
                                            mybir.AluOpType.add)
                nc.sync.dma_start(out_T[:, t0 * 128:(t0 + ntl) * 128],
                                  ots[:, :ntl * 128])

    nc.compile()
    return nc


def _load_sh_slab(nc, c, shp, dest, t0, ntl):
    """Load node-major tiles [t0, t0+ntl) from half-split sh into dest."""
    if t0 + ntl <= c.HT:
        nc.sync.dma_start(dest[:, :ntl, :], shp[0][:, t0:t0 + ntl, :])
    elif t0 >= c.HT:
        nc.sync.dma_start(dest[:, :ntl, :],
                          shp[1][:, t0 - c.HT:t0 - c.HT + ntl, :])
    else:
        n1 = c.HT - t0
        nc.sync.dma_start(dest[:, :n1, :], shp[0][:, t0:c.HT, :])
        nc.sync.dma_start(dest[:, n1:ntl, :], shp[1][:, 0:ntl - n1, :])


# ----------------------------------------------------------------------
# host wrapper
# ----------------------------------------------------------------------

def _prep_inputs(cfg, inputs):
    c = cfg
    x = np.asarray(inputs["x"], np.float32)
    label = np.asarray(inputs["label"], np.int64)
    p = np.asarray(inputs["p"], np.float32)
    cm = np.asarray(inputs["cm"], np.float32)
    lin0_w = np.asarray(inputs["lin0_w"], np.float32)
    lin0_b = np.asarray(inputs["lin0_b"], np.float32)
    lin1_w = np.asarray(inputs["lin1_w"], np.float32)
    lin1_b = np.asarray(inputs["lin1_b"], np.float32)
    conv_w = np.asarray(inputs["conv_w"], np.float32)

    gidx, dstv, wv, slots = _prep_edges(cfg, inputs["edge_index"],
                                        inputs["edge_weight"])

    cnt = np.bincount(label, minlength=c.C).astype(np.float32)
    cnt = np.maximum(cnt, 1.0)
    cma = cm[:, 0, :] * (c.rsl * (1.0 - c.alpha))      # 0.45 fold
    i47 = np.eye(c.C, dtype=np.float32)
    ident = np.eye(128, dtype=np.float32)
    iota = np.tile(np.arange(c.SW, dtype=np.float16)[None, :], (128, 1))
    wceff = np.zeros((c.L, c.H, c.H), np.float32)
    for i in range(c.L):
        beta = float(np.log(c.theta / (i + 1) + 1.0))
        wceff[i] = (1.0 - beta) * np.eye(c.H, dtype=np.float32) \
            + beta * conv_w[i]

    perm = _node_perm(c)
    pidx = perm[np.arange(c.NS)]
    in_maps = []
    for ci in range(c.ncores):
        r0 = ci * c.NS
        xs = np.zeros((c.NSP, c.H), np.float32)
        xs[pidx] = x[r0:r0 + c.NS]
        lab = label[r0:r0 + c.NS]
        d_t = np.zeros((c.NSP, c.C), np.float32)
        d_t[pidx, lab] = 1.0 / cnt[lab]
        p_pad = np.zeros((c.NSP, c.C), np.float32)
        p_pad[pidx] = p[r0:r0 + c.NS]
        in_maps.append({
            "xin_T": np.ascontiguousarray(xs.T).reshape(
                c.H, c.NT, 128).astype(np.float16),
            "d_T": np.ascontiguousarray(
                d_t.reshape(c.NT, 128, c.C).transpose(1, 0, 2)),
            "p_T": np.ascontiguousarray(p_pad.T).astype(np.float16),
            "gidx": gidx[ci], "dstv": dstv[ci], "wv": wv[ci],
            "lin0w": lin0_w, "lin0b": lin0_b.reshape(-1, 1),
            "lin1w": lin1_w, "lin1b": lin1_b.reshape(-1, 1),
            "wceff": wceff, "cma45": cma,
            "cmat45": np.ascontiguousarray(cma.T),
            "i47": i47, "ident": ident, "iota": iota,
        })
    return in_maps, slots


_BUILT = {}


def kernel(**inputs):
    cfg = DEF
    in_maps, slots = _prep_inputs(cfg, inputs)
    key = "default"
    if key not in _BUILT:
        _BUILT[key] = build_nc(cfg, slots)
    nc = _BUILT[key]
    res = bass_utils.run_bass_kernel_spmd(nc, in_maps,
                                          core_ids=list(range(cfg.ncores)))
    pidx = _node_perm(cfg)[np.arange(cfg.NS)]
    outs = [res.results[ci]["out_T"].T[pidx] for ci in range(cfg.ncores)]
    return np.ascontiguousarray(np.concatenate(outs, 0).astype(np.float32))


# revision 18
# speedup vs baseline: 3.3007x; 1.0507x over previous
"""GCN2 (nn_GCN2_42331197669873) Bass kernel for 8 TRN2 NeuronCores.

Strategy: graph/data parallel, nodes sharded row-wise (12544 padded rows
per core).  The sparse propagate is a one-hot-matmul segment sum: edge
source rows are gathered (SWDGE dma_gather spread over 4 queues = 4 Q7
core pairs, ~3.1 ns/token vs 8.7 single-queue) as fp16 rows into SBUF
token groups of 128; each group is multiplied on the tensor engine by an
on-the-fly selection matrix S[tok, dst] = w * (dst_off(tok)==col),
accumulating into a per-supertile PSUM bank [H, 512].  No scatter-add,
no HBM aggregation buffers, f32 accumulation.  The dense combine
(class-center term, 0.45x + 0.1x0, beta-folded conv) continues in the
same PSUM bank via matmul injections, so there is one PSUM round trip
per tile per layer.

All x-valued tensors are fp16: the class centers are nearly identical
(||c|| ~ 28x the pairwise distances), so the normalized class-difference
term amplifies center errors ~8x and bf16 node features are not accurate
enough.  fp16 also keeps the S-build is_equal compare exact (integers
<= 2048).

Node features are replicated per layer with two half AllGathers
(double-buffered x_rep so the next layer's AG overlaps this layer's
tail).  Weights/class tensors are tiny and resident.

kernel(**inputs) takes the FULL unsharded inputs and returns the FULL
[100000, 47] output; sharding + preprocessing happens on host inside.
"""

import numpy as np
import ml_dtypes

from concourse import bass, bacc, tile, mybir, bass_utils
from concourse import library_config
from concourse.mybir import AxisListType
import concourse.tile_sem_assignment as _tsa
from concourse import bass_isa as _bisa

NQUEUES = 4
ACT_MOD = 2          # every ACT_MOD-th S-build goes to the ACT engine (0=off)

# Tile round-robins Pool-engine DMAs over all DMASW sem lanes ignoring
# queue_num; mixing SWDGE queues on one lane breaks its in-order-completion
# assumption (sim: "sem locked to SWDGE queue"). Segregate lanes by queue:
# queue q -> lanes [2q, 2q+1].
_orig_assign_tick = _tsa.TileClockTick._assign_tick


def _assign_tick_qsplit(self, inst):
    if (isinstance(inst, _tsa.DMAInst)
            and inst.engine == mybir.EngineType.Pool
            and not isinstance(inst, _bisa.UserSyncedRemoteDMADescs)
            and self.swdge_sem_count >= NQUEUES * 2):
        qn = getattr(inst, "queue_num", 0) or 0
        lanes = self.swdge_sem_count // NQUEUES
        if not hasattr(self, "_qrr"):
            self._qrr = {}
        r = self._qrr.get(qn, 0)
        self._qrr[qn] = r + 1
        self.next_sw_dma_idx = (qn % NQUEUES) * lanes + r % lanes
    return _orig_assign_tick(self, inst)


_tsa.TileClockTick._assign_tick = _assign_tick_qsplit

F32 = mybir.dt.float32
BF16 = mybir.dt.bfloat16
FP16 = mybir.dt.float16
I16 = mybir.dt.int16
AF = mybir.ActivationFunctionType


class Cfg:
    def __init__(self):
        self.N, self.E, self.C, self.H = 100000, 800000, 47, 128
        self.ncores = 8
        self.NS = self.N // self.ncores      # real nodes per core
        self.NT = 98                         # 128-row tiles per core
        self.NSP = self.NT * 128             # padded nodes per core
        self.HT = 49                         # tiles per half
        self.HR = self.HT * 128              # rows per half shard (6272)
        self.REG = self.ncores * self.HR     # x_rep region rows (50176)
        self.W = self.REG // 2               # gather window rows (25088)
        self.NW = 4                          # windows (2 per region)
        self.STW = 512                       # supertile width (psum bank)
        self.NST = (self.NSP + 511) // 512   # supertiles per core (25)
        self.NRES = 2                        # node interleave classes
        self.SW = self.STW // self.NRES      # S matrix width (256)
        self.CHST = 4                        # supertiles per chunk
        self.NCH = (self.NST + 3) // 4       # chunks (7)
        self.L = 4
        self.alpha, self.theta, self.rsl = 0.1, 0.5, 0.5


DEF = Cfg()


# ----------------------------------------------------------------------
# host-side edge preprocessing
# ----------------------------------------------------------------------

def _node_perm(cfg):
    """Physical position of each padded local node index (within-supertile
    interleave by residue class, so S matrices are STW/NRES wide)."""
    c = cfg
    r = np.arange(c.NSP)
    st = r // c.STW
    stw = np.minimum(c.STW, c.NSP - st * c.STW)
    sw = stw // c.NRES
    w = r - st * c.STW
    return st * c.STW + (w % c.NRES) * sw + w // c.NRES


def _prep_edges(cfg, edge_index, edge_weight):
    """Token layout: cells (supertile st, residue rr, window w); slots =
    128-multiple max-over-cores capacity; tokens sorted by gather idx,
    trailing pads use idx -1 (trimmed by the Q7 per core).

    Returns gidx [nc,128,COLS] i16, dstv/wv [nc,128,NB] f32,
    slots [NST][NRES][NW].
    """
    c = cfg
    perm = _node_perm(c)
    src = np.asarray(edge_index[0], np.int64)
    dst = np.asarray(edge_index[1], np.int64)
    w = np.asarray(edge_weight, np.float32) * (1.0 - c.rsl) * (1.0 - c.alpha)
    nc_, NS = c.ncores, c.NS

    core = dst // NS
    r = dst - core * NS
    st = r // c.STW
    stw = np.minimum(c.STW, c.NSP - st * c.STW)
    sw = stw // c.NRES
    within = r - st * c.STW
    rr = within % c.NRES
    qq = within // c.NRES

    cs, rs = src // NS, src % NS
    rp = perm[rs]
    ts, ps = rp // 128, rp % 128
    reg = (ts >= c.HT).astype(np.int64)
    row = cs * c.HR + ps * c.HT + (ts - c.HT * reg)
    win = 2 * reg + row // c.W
    gix = row - (row // c.W) * c.W

    cnt = np.zeros((nc_, c.NST, c.NRES, c.NW), np.int64)
    np.add.at(cnt, (core, st, rr, win), 1)
    cap = cnt.max(axis=0)
    slots = (cap + 127) // 128

    nb = int(slots.sum())
    TOK = nb * 128
    gidx = np.zeros((nc_, 128, TOK // 16), np.int16)
    dstv = np.full((nc_, 128, nb), -1.0, np.float32)
    wv = np.zeros((nc_, 128, nb), np.float32)

    cell_g0 = np.zeros((c.NST, c.NRES, c.NW), np.int64)
    g = 0
    for s in range(c.NST):
        for e in range(c.NRES):
            for b in range(c.NW):
                cell_g0[s, e, b] = g
                g += slots[s, e, b]

    ti = np.arange(TOK)
    rows16 = (ti % 16)[None, :] + 16 * np.arange(8)[:, None]
    cols16 = ti // 16

    key = ((core * c.NST + st) * c.NRES + rr) * c.NW + win
    order = np.lexsort((gix, key))
    ksort = key[order]
    nkey = nc_ * c.NST * c.NRES * c.NW
    bounds = np.searchsorted(ksort, np.arange(nkey + 1))
    for ci in range(nc_):
        g_lin = np.zeros(TOK, np.int16)
        d_lin = np.full(TOK, -1.0, np.float32)
        w_lin = np.zeros(TOK, np.float32)
        for s in range(c.NST):
            for e in range(c.NRES):
                for b in range(c.NW):
                    kk = ((ci * c.NST + s) * c.NRES + e) * c.NW + b
                    sel = order[bounds[kk]:bounds[kk + 1]]
                    n = len(sel)
                    t0 = int(cell_g0[s, e, b]) * 128
                    g_lin[t0:t0 + n] = gix[sel].astype(np.int16)
                    d_lin[t0:t0 + n] = qq[sel].astype(np.float32)
                    w_lin[t0:t0 + n] = w[sel]
                    ns_ = int(slots[s, e, b])
                    for q0_ in range(0, ns_, 8):
                        pos = t0 + q0_ * 128
                        if g_lin[pos] < 0:
                            g_lin[pos] = 0
        for gg in range(8):
            gidx[ci, rows16[gg], cols16] = g_lin
        dstv[ci, ti % 128, ti // 128] = d_lin
        wv[ci, ti % 128, ti // 128] = w_lin
    return gidx, dstv, wv, slots


# ----------------------------------------------------------------------
# device program
# ----------------------------------------------------------------------

def build_nc(cfg, slots):
    c = cfg
    nc = bacc.Bacc(None, target_bir_lowering=False, debug=False,
                   num_swdge_queues=NQUEUES)
    NT, NSP, C_, H = c.NT, c.NSP, c.C, c.H
    L = c.L
    NB = int(slots.sum())
    TOK = NB * 128

    def dram_in(name, shape, dt=F32):
        return nc.declare_dram_parameter(name, shape, dt, isOutput=False)

    xin_T = dram_in("xin_T", [H, NT, 128], FP16)
    d_T = dram_in("d_T", [128, NT, C_], F32)
    p_T = dram_in("p_T", [C_, NSP], FP16)
    gidx = dram_in("gidx", [128, TOK // 16], I16)
    dstv = dram_in("dstv", [128, NB], F32)
    wv = dram_in("wv", [128, NB], F32)
    lin0w = dram_in("lin0w", [H, H])
    lin0b = dram_in("lin0b", [H, 1])
    lin1w = dram_in("lin1w", [H, C_])
    lin1b = dram_in("lin1b", [C_, 1])
    wceff = dram_in("wceff", [L, H, H])
    cma45 = dram_in("cma45", [C_, C_])
    cmat45 = dram_in("cmat45", [C_, C_])
    i47 = dram_in("i47", [C_, C_])
    ident = dram_in("ident", [128, 128])
    iota = dram_in("iota", [128, c.SW], FP16)
    out_T = nc.declare_dram_parameter("out_T", [C_, NSP], F32, isOutput=True)

    # internal DRAM
    x_rep = [[nc.dram_tensor(f"x_rep{h}_{pbuf}", [c.REG, H], FP16,
                             addr_space="Shared")
              for pbuf in range(2)] for h in range(2)]
    sh = [[nc.dram_tensor(f"sh{i}{'AB'[h]}", [128, c.HT, H], FP16)
           for h in range(2)] for i in range(3)]
    # feature-major fp16 x buffers; xTh[0] holds x0 and is never overwritten
    xTh = [nc.dram_tensor(f"xTh{i}", [128, NT, H], FP16) for i in range(3)]
    cen_in = nc.dram_tensor("cen_in", [C_, H], F32)
    cen_out = nc.dram_tensor("cen_out", [C_, H], F32, addr_space="Shared")

    rg = [list(range(c.ncores))]
    CURS = [0, 1, 2, 1]
    NXTS = [1, 2, 1, 2]

    cell_g0 = np.zeros((c.NST, c.NRES, c.NW), np.int64)
    g = 0
    for s in range(c.NST):
        for e in range(c.NRES):
            for b in range(c.NW):
                cell_g0[s, e, b] = g
                g += slots[s, e, b]

    with tile.TileContext(nc) as tc:
        nc.gpsimd.load_library(library_config.mlp)
        with (
            tc.tile_pool(name="const", bufs=1) as cpool,
            tc.tile_pool(name="edge", bufs=1) as epool,
            tc.tile_pool(name="bslab", bufs=2) as bpool,
            tc.tile_pool(name="oslab", bufs=2) as opool,
            tc.tile_pool(name="gt", bufs=8) as gpool,
            tc.tile_pool(name="smat", bufs=6) as spool,
            tc.tile_pool(name="sb", bufs=4) as pool,
            tc.tile_pool(name="bank", bufs=c.CHST, space="PSUM") as bankp,
            tc.tile_pool(name="ps2", bufs=2, space="PSUM") as ps2p,
            tc.tile_pool(name="psn", bufs=1, space="PSUM") as psnp,
            tc.tile_pool(name="psA", bufs=1, space="PSUM") as psAp,
        ):
            # ---- resident constants ----
            lin0w_sb = cpool.tile([H, H], F32)
            nc.sync.dma_start(lin0w_sb[:], lin0w[:, :])
            lin0w_h = cpool.tile([H, H], FP16)
            nc.vector.tensor_copy(lin0w_h[:], lin0w_sb[:])
            lin0b_sb = cpool.tile([H, 1], F32)
            nc.sync.dma_start(lin0b_sb[:], lin0b[:, :])
            lin1w_sb = cpool.tile([H, C_], F32)
            nc.sync.dma_start(lin1w_sb[:], lin1w[:, :])
            lin1w_h = cpool.tile([H, C_], FP16)
            nc.vector.tensor_copy(lin1w_h[:], lin1w_sb[:])
            lin1b_sb = cpool.tile([C_, 1], F32)
            nc.sync.dma_start(lin1b_sb[:], lin1b[:, :])
            wceff_sb = cpool.tile([H, L * H], F32)
            for i in range(L):
                nc.sync.dma_start(wceff_sb[:, i * H:(i + 1) * H], wceff[i])
            cma_sb = cpool.tile([C_, C_], F32)
            nc.sync.dma_start(cma_sb[:], cma45[:, :])
            cmat_sb = cpool.tile([C_, C_], F32)
            nc.sync.dma_start(cmat_sb[:], cmat45[:, :])
            i47_sb = cpool.tile([C_, C_], F32)
            nc.sync.dma_start(i47_sb[:], i47[:, :])
            ident_sb = cpool.tile([128, 128], F32)
            nc.sync.dma_start(ident_sb[:], ident[:, :])
            identh_sb = cpool.tile([128, 128], FP16)
            nc.vector.tensor_copy(identh_sb[:], ident_sb[:])
            i45h_sb = cpool.tile([128, 128], FP16)
            nc.vector.tensor_scalar(i45h_sb[:], ident_sb[:], 0.45, None,
                                    mybir.AluOpType.mult)
            i10h_sb = cpool.tile([128, 128], FP16)
            nc.vector.tensor_scalar(i10h_sb[:], ident_sb[:], 0.1, None,
                                    mybir.AluOpType.mult)
            iota_sb = cpool.tile([128, c.SW], FP16)
            nc.sync.dma_start(iota_sb[:], iota[:, :])

            # ---- resident edge data ----
            gi_sb = epool.tile([128, TOK // 16], I16)
            nc.sync.dma_start(gi_sb[:], gidx[:, :])
            dv_sb = epool.tile([128, NB], F32)
            nc.sync.dma_start(dv_sb[:], dstv[:, :])
            wv_sb = epool.tile([128, NB], F32)
            nc.sync.dma_start(wv_sb[:], wv[:, :])
            d_res = epool.tile([128, NT, C_], F32)
            nc.sync.dma_start(d_res[:], d_T[:, :, :])
            cen0_sb = epool.tile([C_, H], F32)
            if ACT_MOD:
                # aux for ACT-engine S-build: t=Square((iota-d)/32),
                # S=Relu(w - 4096*w*t)
                dvn_sb = epool.tile([128, NB], F32)
                nc.vector.tensor_scalar(dvn_sb[:], dv_sb[:], -1.0 / 32, None,
                                        mybir.AluOpType.mult)
                wvn_sb = epool.tile([128, NB], F32)
                nc.vector.tensor_scalar(wvn_sb[:], wv_sb[:], -4096.0, None,
                                        mybir.AluOpType.mult)

            CHT = c.CHST * 4                       # tiles per chunk (16)

            def chunk_tiles(ch):
                t0 = ch * CHT
                return t0, min(CHT, NT - t0)

            def write_sh(dst_sh, shs, t0, ntl):
                if t0 + ntl <= c.HT:
                    nc.sync.dma_start(dst_sh[0][:, t0:t0 + ntl, :],
                                      shs[:, :ntl, :])
                elif t0 >= c.HT:
                    nc.sync.dma_start(
                        dst_sh[1][:, t0 - c.HT:t0 - c.HT + ntl, :],
                        shs[:, :ntl, :])
                else:
                    n1 = c.HT - t0
                    nc.sync.dma_start(dst_sh[0][:, t0:c.HT, :],
                                      shs[:, :n1, :])
                    nc.sync.dma_start(dst_sh[1][:, 0:ntl - n1, :],
                                      shs[:, n1:ntl, :])

            # ---- lin0 (also accumulates psA0 = d^T x0 in f32) ----
            psA = psAp.tile([C_, H], F32, tag="cen")
            for ch in range(c.NCH):
                t0, ntl = chunk_tiles(ch)
                xi = bpool.tile([128, CHT, H], FP16, tag="xTs")
                nc.sync.dma_start(xi[:, :ntl, :], xin_T[:, t0:t0 + ntl, :])
                xhs = opool.tile([128, CHT, H], FP16, tag="xhs")
                shs = opool.tile([128, CHT, H], FP16, tag="shs")
                for j in range(ntl):
                    t = t0 + j
                    ps0 = ps2p.tile([H, 128], F32, tag="b")
                    nc.tensor.matmul(ps0[:], lin0w_h[:], xi[:, j, :],
                                     start=True, stop=True)
                    nc.scalar.activation(xhs[:, j, :], ps0[:], AF.Relu,
                                         bias=lin0b_sb[:, 0:1])
                    xf = pool.tile([H, 128], F32, tag="xf")
                    nc.scalar.activation(xf[:], ps0[:], AF.Relu,
                                         bias=lin0b_sb[:, 0:1])
                    psn = psnp.tile([128, 128], F32, tag="bb")
                    nc.tensor.transpose(psn[:], xf[:], ident_sb[:])
                    nc.vector.tensor_copy(shs[:, j, :], psn[:])
                    sf = pool.tile([128, H], F32, tag="sf")
                    nc.vector.tensor_copy(sf[:], psn[:])
                    nc.tensor.matmul(psA[:], d_res[:, t, :], sf[:],
                                     start=(t == 0), stop=(t == NT - 1))
                nc.sync.dma_start(xTh[0][:, t0:t0 + ntl, :], xhs[:, :ntl, :])
                write_sh(sh[0], shs, t0, ntl)
            nc.vector.tensor_copy(cen0_sb[:], psA[:])

            # ---- layers ----
            qrr = [0]

            for li in range(L):
                cur, nxt = CURS[li], NXTS[li]
                pb = li % 2
                for h in range(2):
                    nc.gpsimd.collective_compute(
                        "AllGather", mybir.AluOpType.bypass,
                        replica_groups=rg,
                        ins=[sh[cur][h].ap().opt()],
                        outs=[x_rep[h][pb].ap().opt()],
                    )

                cenp = pool.tile([C_, H], F32, tag="cenp")
                if li == 0:
                    nc.vector.tensor_scalar(cenp[:], cen0_sb[:], 1.1, None,
                                            mybir.AluOpType.mult)
                else:
                    nc.vector.tensor_scalar(cenp[:], cen0_sb[:], 0.1, None,
                                            mybir.AluOpType.mult)
                    nc.vector.tensor_add(cenp[:], cenp[:], psA[:])
                nc.sync.dma_start(cen_in[:, :], cenp[:])
                nc.gpsimd.collective_compute(
                    "AllReduce", mybir.AluOpType.add, replica_groups=rg,
                    ins=[cen_in.ap().opt()], outs=[cen_out.ap().opt()],
                )
                cen = pool.tile([C_, H], F32, tag="cen_sb")
                nc.sync.dma_start(cen[:], cen_out[:, :])

                # - r_cls from centers (Gram trick), cma pre-scaled 0.45 -
                pst = ps2p.tile([128, C_], F32, tag="b")
                nc.tensor.transpose(pst[:, :], cen[:], ident_sb[:C_, :C_])
                cT = pool.tile([128, C_], F32, tag="cT")
                nc.vector.tensor_copy(cT[:], pst[:, :])
                psg = ps2p.tile([C_, C_], F32, tag="b")
                nc.tensor.matmul(psg[:], cT[:], cT[:], start=True, stop=True)
                gg = pool.tile([C_, C_], F32, tag="gg")
                nc.vector.tensor_copy(gg[:], psg[:])
                gd = pool.tile([C_, C_], F32, tag="gd")
                nc.vector.tensor_mul(gd[:], gg[:], i47_sb[:])
                n2 = pool.tile([C_, 1], F32, tag="n2")
                nc.vector.reduce_sum(n2[:], gd[:], AxisListType.X)
                t1 = pool.tile([C_, C_], F32, tag="t1")
                nc.vector.tensor_scalar(t1[:], gg[:], -1.0, n2[:, 0:1],
                                        mybir.AluOpType.mult,
                                        mybir.AluOpType.add)
                ps1 = ps2p.tile([C_, C_], F32, tag="b")
                nc.tensor.transpose(ps1[:], t1[:], ident_sb[:C_, :C_])
                nrm = pool.tile([C_, C_], F32, tag="nrm")
                nc.vector.tensor_add(nrm[:], t1[:], ps1[:])
                nc.vector.tensor_relu(nrm[:], nrm[:])
                nc.vector.tensor_add(nrm[:], nrm[:], i47_sb[:])
                rn = pool.tile([C_, C_], F32, tag="rn")
                nc.scalar.sqrt(rn[:], nrm[:])
                inv = pool.tile([C_, C_], F32, tag="inv")
                nc.vector.reciprocal(inv[:], rn[:])
                amat = pool.tile([C_, C_], F32, tag="amat")
                nc.vector.tensor_mul(amat[:], cma_sb[:], inv[:])
                atm = pool.tile([C_, C_], F32, tag="atm")
                nc.vector.tensor_mul(atm[:], cmat_sb[:], inv[:])
                rs = pool.tile([C_, 1], F32, tag="rs")
                nc.vector.reduce_sum(rs[:], amat[:], AxisListType.X)
                psm = ps2p.tile([C_, H], F32, tag="b")
                nc.tensor.matmul(psm[:], atm[:], cen[:], start=True, stop=True)
                rcls = pool.tile([C_, H], F32, tag="rcls")
                nc.vector.tensor_scalar(rcls[:], cen[:], rs[:, 0:1], None,
                                        mybir.AluOpType.mult)
                nc.vector.tensor_sub(rcls[:], rcls[:], psm[:])
                rclsh = pool.tile([C_, H], FP16, tag="rclsh")
                nc.vector.tensor_copy(rclsh[:], rcls[:])

                # - propagate + pass B, chunked -
                for ch in range(c.NCH):
                    t0, ntl = chunk_tiles(ch)
                    st0 = ch * c.CHST
                    nst = min(c.CHST, c.NST - st0)
                    xTs = bpool.tile([128, CHT, H], FP16, tag="xTs")
                    nc.sync.dma_start(xTs[:, :ntl, :],
                                      xTh[cur][:, t0:t0 + ntl, :])
                    x0s2 = bpool.tile([128, CHT, H], FP16, tag="x0s2")
                    nc.sync.dma_start(x0s2[:, :ntl, :],
                                      xTh[0][:, t0:t0 + ntl, :])
                    pts = bpool.tile([C_, CHT * 128], FP16, tag="pts")
                    nc.sync.dma_start(pts[:, :ntl * 128],
                                      p_T[:, t0 * 128:(t0 + ntl) * 128])

                    banks = []
                    for si in range(nst):
                        st = st0 + si
                        stw_st = min(c.STW, NSP - st * c.STW)
                        sw_st = stw_st // c.NRES
                        bank = bankp.tile([H, c.STW], F32, tag="bank")
                        banks.append(bank)
                        first = [True]
                        jlast = min(3, NT - 1 - st * 4)
                        for e, wnd in [(e_, w_) for w01 in (0, 1)
                                       for e_ in range(c.NRES)
                                       for w_ in (2 * w01, 2 * w01 + 1)]:
                            bsl = bank[:, e * sw_st:(e + 1) * sw_st]
                            if True:
                                g0 = int(cell_g0[st, e, wnd])
                                ng = int(slots[st, e, wnd])
                                if ng == 0:
                                    continue
                                rep = x_rep[wnd // 2][pb]
                                base = (wnd % 2) * c.W
                                gts = []
                                for q0 in range(0, ng, 8):
                                    q1 = min(q0 + 8, ng)
                                    ntok = (q1 - q0) * 128
                                    gt = gpool.tile([128, 8, H], FP16,
                                                    tag="g")
                                    nc.gpsimd.dma_gather(
                                        gt[:, :q1 - q0, :],
                                        rep[base:base + c.W, :],
                                        gi_sb[:, (g0 + q0) * 8:(g0 + q1) * 8],
                                        num_idxs=ntok, num_idxs_reg=ntok,
                                        elem_size=H,
                                        queue_num=qrr[0] % NQUEUES,
                                    )
                                    qrr[0] += 1
                                    gts.append((gt, q0, q1 - q0))
                                for (gt, q0, nq) in gts:
                                    for k in range(nq):
                                        gl = g0 + q0 + k
                                        S = spool.tile([128, c.SW], FP16,
                                                       tag="S")
                                        if ACT_MOD and gl % ACT_MOD == 0:
                                            St = spool.tile([128, c.SW],
                                                            FP16, tag="St")
                                            nc.scalar.activation(
                                                St[:], iota_sb[:], AF.Square,
                                                bias=dvn_sb[:, gl:gl + 1],
                                                scale=1.0 / 32)
                                            nc.scalar.activation(
                                                S[:], St[:], AF.Relu,
                                                bias=wv_sb[:, gl:gl + 1],
                                                scale=wvn_sb[:, gl:gl + 1])
                                        else:
                                            nc.vector.tensor_scalar(
                                                S[:], iota_sb[:],
                                                dv_sb[:, gl:gl + 1],
                                                wv_sb[:, gl:gl + 1],
                                                mybir.AluOpType.is_equal,
                                                mybir.AluOpType.mult)
                                        nc.tensor.matmul(
                                            bsl, gt[:, k, :], S[:, :sw_st],
                                            start=first[0], stop=False,
                                            skip_group_check=True)
                                        first[0] = False
                        for j in range(4):
                            t = st * 4 + j
                            if t >= NT:
                                break
                            jj = t - t0
                            sl = bank[:, j * 128:(j + 1) * 128]
                            nc.tensor.matmul(sl, rclsh[:],
                                             pts[:, jj * 128:(jj + 1) * 128],
                                             start=first[0], stop=False,
                                             skip_group_check=True)
                            nc.tensor.matmul(sl, i45h_sb[:], xTs[:, jj, :],
                                             start=False, stop=False,
                                             skip_group_check=True)
                            nc.tensor.matmul(sl, i10h_sb[:], x0s2[:, jj, :],
                                             start=False, stop=(j == jlast),
                                             skip_group_check=True)
                            first[0] = False

                    # pass B compute per tile (also accumulates next
                    # layer's centers from f32 data); the last layer
                    # feeds lin1 directly instead of storing x
                    if ch == 0 and li < L - 1:
                        psA = psAp.tile([C_, H], F32, tag="cen")
                    if li < L - 1:
                        xhs = opool.tile([128, CHT, H], FP16, tag="xhs")
                        shs = opool.tile([128, CHT, H], FP16, tag="shs")
                    else:
                        ots = opool.tile([C_, CHT * 128], F32, tag="ots")
                    for si in range(nst):
                        st = st0 + si
                        for j in range(4):
                            t = st * 4 + j
                            if t >= NT:
                                break
                            jj = t - t0
                            u = pool.tile([H, 128], F32, tag="u")
                            nc.vector.tensor_copy(
                                u[:], banks[si][:, j * 128:(j + 1) * 128])
                            ps2 = ps2p.tile([H, 128], F32, tag="b")
                            nc.tensor.matmul(ps2[:],
                                             wceff_sb[:, li * H:(li + 1) * H],
                                             u[:], start=True, stop=True)
                            if li < L - 1:
                                nc.scalar.activation(xhs[:, jj, :], ps2[:],
                                                     AF.Relu)
                                xf = pool.tile([H, 128], F32, tag="xf")
                                nc.scalar.activation(xf[:], ps2[:], AF.Relu)
                                psn = psnp.tile([128, 128], F32, tag="bb")
                                nc.tensor.transpose(psn[:], xf[:],
                                                    ident_sb[:])
                                nc.vector.tensor_copy(shs[:, jj, :], psn[:])
                                sf = pool.tile([128, H], F32, tag="sf")
                                nc.vector.tensor_copy(sf[:], psn[:])
                                nc.tensor.matmul(psA[:], d_res[:, t, :],
                                                 sf[:], start=(t == 0),
                                                 stop=(t == NT - 1))
                            else:
                                xh = pool.tile([H, 128], FP16, tag="xh")
                                nc.scalar.activation(xh[:], ps2[:], AF.Relu)
                                psf = ps2p.tile([C_, 128], F32, tag="b")
                                nc.tensor.matmul(psf[:], lin1w_h[:], xh[:],
                                                 start=True, stop=True)
                                nc.vector.tensor_scalar(
                                    ots[:, jj * 128:(jj + 1) * 128],
                                    psf[:], lin1b_sb[:, 0:1], None,
                                    mybir.AluOpType.add)
                    if li < L - 1:
                        nc.sync.dma_start(xTh[nxt][:, t0:t0 + ntl, :],
                                          xhs[:, :ntl, :])
                        write_sh(sh[nxt], shs, t0, ntl)
                    else:
                        nc.sync.dma_start(
                            out_T[:, t0 * 128:(t0 + ntl) * 128],
                            ots[:, :ntl * 128])

    nc.compile()
    return nc


def _load_sh_slab(nc, c, shp, dest, t0, ntl):
    """Load node-major tiles [t0, t0+ntl) from half-split sh into dest."""
    if t0 + ntl <= c.HT:
        nc.sync.dma_start(dest[:, :ntl, :], shp[0][:, t0:t0 + ntl, :])
    elif t0 >= c.HT:
        nc.sync.dma_start(dest[:, :ntl, :],
                          shp[1][:, t0 - c.HT:t0 - c.HT + ntl, :])
    else:
        n1 = c.HT - t0
        nc.sync.dma_start(dest[:, :n1, :], shp[0][:, t0:c.HT, :])
        nc.sync.dma_start(dest[:, n1:ntl, :], shp[1][:, 0:ntl - n1, :])


# ----------------------------------------------------------------------
# host wrapper
# ----------------------------------------------------------------------

def _prep_inputs(cfg, inputs):
    c = cfg
    x = np.asarray(inputs["x"], np.float32)
    label = np.asarray(inputs["label"], np.int64)
    p = np.asarray(inputs["p"], np.float32)
    cm = np.asarray(inputs["cm"], np.float32)
    lin0_w = np.asarray(inputs["lin0_w"], np.float32)
    lin0_b = np.asarray(inputs["lin0_b"], np.float32)
    lin1_w = np.asarray(inputs["lin1_w"], np.float32)
    lin1_b = np.asarray(inputs["lin1_b"], np.float32)
    conv_w = np.asarray(inputs["conv_w"], np.float32)

    gidx, dstv, wv, slots = _prep_edges(cfg, inputs["edge_index"],
                                        inputs["edge_weight"])

    cnt = np.bincount(label, minlength=c.C).astype(np.float32)
    cnt = np.maximum(cnt, 1.0)
    cma = cm[:, 0, :] * (c.rsl * (1.0 - c.alpha))      # 0.45 fold
    i47 = np.eye(c.C, dtype=np.float32)
    ident = np.eye(128, dtype=np.float32)
    iota = np.tile(np.arange(c.SW, dtype=np.float16)[None, :], (128, 1))
    wceff = np.zeros((c.L, c.H, c.H), np.float32)
    for i in range(c.L):
        beta = float(np.log(c.theta / (i + 1) + 1.0))
        wceff[i] = (1.0 - beta) * np.eye(c.H, dtype=np.float32) \
            + beta * conv_w[i]

    perm = _node_perm(c)
    pidx = perm[np.arange(c.NS)]
    in_maps = []
    for ci in range(c.ncores):
        r0 = ci * c.NS
        xs = np.zeros((c.NSP, c.H), np.float32)
        xs[pidx] = x[r0:r0 + c.NS]
        lab = label[r0:r0 + c.NS]
        d_t = np.zeros((c.NSP, c.C), np.float32)
        d_t[pidx, lab] = 1.0 / cnt[lab]
        p_pad = np.zeros((c.NSP, c.C), np.float32)
        p_pad[pidx] = p[r0:r0 + c.NS]
        in_maps.append({
            "xin_T": np.ascontiguousarray(xs.T).reshape(
                c.H, c.NT, 128).astype(np.float16),
            "d_T": np.ascontiguousarray(
                d_t.reshape(c.NT, 128, c.C).transpose(1, 0, 2)),
            "p_T": np.ascontiguousarray(p_pad.T).astype(np.float16),
            "gidx": gidx[ci], "dstv": dstv[ci], "wv": wv[ci],
            "lin0w": lin0_w, "lin0b": lin0_b.reshape(-1, 1),
            "lin1w": lin1_w, "lin1b": lin1_b.reshape(-1, 1),
            "wceff": wceff, "cma45": cma,
            "cmat45": np.ascontiguousarray(cma.T),
            "i47": i47, "ident": ident, "iota": iota,
        })
    return in_maps, slots


_BUILT = {}


def kernel(**inputs):
    cfg = DEF
    in_maps, slots = _prep_inputs(cfg, inputs)
    key = "default"
    if key not in _BUILT:
        _BUILT[key] = build_nc(cfg, slots)
    nc = _BUILT[key]
    res = bass_utils.run_bass_kernel_spmd(nc, in_maps,
                                          core_ids=list(range(cfg.ncores)))
    pidx = _node_perm(cfg)[np.arange(cfg.NS)]
    outs = [res.results[ci]["out_T"].T[pidx] for ci in range(cfg.ncores)]
    return np.ascontiguousarray(np.concatenate(outs, 0).astype(np.float32))


# revision 19
# speedup vs baseline: 3.4183x; 1.0356x over previous
"""GCN2 (nn_GCN2_42331197669873) Bass kernel for 8 TRN2 NeuronCores.

Strategy: graph/data parallel, nodes sharded row-wise (12544 padded rows
per core).  The sparse propagate is a one-hot-matmul segment sum: edge
source rows are gathered (SWDGE dma_gather spread over 4 queues = 4 Q7
core pairs, ~3.1 ns/token vs 8.7 single-queue) as fp16 rows into SBUF
token groups of 128; each group is multiplied on the tensor engine by an
on-the-fly selection matrix S[tok, dst] = w * (dst_off(tok)==col),
accumulating into a per-supertile PSUM bank [H, 512].  No scatter-add,
no HBM aggregation buffers, f32 accumulation.  The dense combine
(class-center term, 0.45x + 0.1x0, beta-folded conv) continues in the
same PSUM bank via matmul injections, so there is one PSUM round trip
per tile per layer.

All x-valued tensors are fp16: the class centers are nearly identical
(||c|| ~ 28x the pairwise distances), so the normalized class-difference
term amplifies center errors ~8x and bf16 node features are not accurate
enough.  fp16 also keeps the S-build is_equal compare exact (integers
<= 2048).

Node features are replicated per layer with two half AllGathers
(double-buffered x_rep so the next layer's AG overlaps this layer's
tail).  Weights/class tensors are tiny and resident.

kernel(**inputs) takes the FULL unsharded inputs and returns the FULL
[100000, 47] output; sharding + preprocessing happens on host inside.
"""

import numpy as np
import ml_dtypes

from concourse import bass, bacc, tile, mybir, bass_utils
from concourse import library_config
from concourse.mybir import AxisListType
import concourse.tile_sem_assignment as _tsa
from concourse import bass_isa as _bisa

NQUEUES = 4
ACT_MOD = 2          # every ACT_MOD-th S-build goes to the ACT engine (0=off)

# Tile round-robins Pool-engine DMAs over all DMASW sem lanes ignoring
# queue_num; mixing SWDGE queues on one lane breaks its in-order-completion
# assumption (sim: "sem locked to SWDGE queue"). Segregate lanes by queue:
# queue q -> lanes [2q, 2q+1].
_orig_assign_tick = _tsa.TileClockTick._assign_tick


def _assign_tick_qsplit(self, inst):
    if (isinstance(inst, _tsa.DMAInst)
            and inst.engine == mybir.EngineType.Pool
            and not isinstance(inst, _bisa.UserSyncedRemoteDMADescs)
            and self.swdge_sem_count >= NQUEUES * 2):
        qn = getattr(inst, "queue_num", 0) or 0
        lanes = self.swdge_sem_count // NQUEUES
        if not hasattr(self, "_qrr"):
            self._qrr = {}
        r = self._qrr.get(qn, 0)
        self._qrr[qn] = r + 1
        self.next_sw_dma_idx = (qn % NQUEUES) * lanes + r % lanes
    return _orig_assign_tick(self, inst)


_tsa.TileClockTick._assign_tick = _assign_tick_qsplit

F32 = mybir.dt.float32
BF16 = mybir.dt.bfloat16
FP16 = mybir.dt.float16
I16 = mybir.dt.int16
AF = mybir.ActivationFunctionType


class Cfg:
    def __init__(self):
        self.N, self.E, self.C, self.H = 100000, 800000, 47, 128
        self.ncores = 8
        self.NS = self.N // self.ncores      # real nodes per core
        self.NT = 98                         # 128-row tiles per core
        self.NSP = self.NT * 128             # padded nodes per core
        self.HT = 49                         # tiles per half
        self.HR = self.HT * 128              # rows per half shard (6272)
        self.REG = self.ncores * self.HR     # x_rep region rows (50176)
        self.W = self.REG // 2               # gather window rows (25088)
        self.NW = 4                          # windows (2 per region)
        self.STW = 512                       # supertile width (psum bank)
        self.NST = (self.NSP + 511) // 512   # supertiles per core (25)
        self.NRES = 2                        # node interleave classes
        self.SW = self.STW // self.NRES      # S matrix width (256)
        self.CHST = 4                        # supertiles per chunk
        self.NCH = (self.NST + 3) // 4       # chunks (7)
        self.L = 4
        self.alpha, self.theta, self.rsl = 0.1, 0.5, 0.5


DEF = Cfg()


# ----------------------------------------------------------------------
# host-side edge preprocessing
# ----------------------------------------------------------------------

def _node_perm(cfg):
    """Physical position of each padded local node index (within-supertile
    interleave by residue class, so S matrices are STW/NRES wide)."""
    c = cfg
    r = np.arange(c.NSP)
    st = r // c.STW
    stw = np.minimum(c.STW, c.NSP - st * c.STW)
    sw = stw // c.NRES
    w = r - st * c.STW
    return st * c.STW + (w % c.NRES) * sw + w // c.NRES


def _prep_edges(cfg, edge_index, edge_weight):
    """Token layout: cells (supertile st, residue rr, window w); slots =
    128-multiple max-over-cores capacity; tokens sorted by gather idx,
    trailing pads use idx -1 (trimmed by the Q7 per core).

    Returns gidx [nc,128,COLS] i16, dstv/wv [nc,128,NB] f32,
    slots [NST][NRES][NW].
    """
    c = cfg
    perm = _node_perm(c)
    src = np.asarray(edge_index[0], np.int64)
    dst = np.asarray(edge_index[1], np.int64)
    w = np.asarray(edge_weight, np.float32) * (1.0 - c.rsl) * (1.0 - c.alpha)
    nc_, NS = c.ncores, c.NS

    core = dst // NS
    r = dst - core * NS
    st = r // c.STW
    stw = np.minimum(c.STW, c.NSP - st * c.STW)
    sw = stw // c.NRES
    within = r - st * c.STW
    rr = within % c.NRES
    qq = within // c.NRES

    cs, rs = src // NS, src % NS
    rp = perm[rs]
    ts, ps = rp // 128, rp % 128
    reg = (ts >= c.HT).astype(np.int64)
    row = cs * c.HR + ps * c.HT + (ts - c.HT * reg)
    win = 2 * reg + row // c.W
    gix = row - (row // c.W) * c.W

    cnt = np.zeros((nc_, c.NST, c.NRES, c.NW), np.int64)
    np.add.at(cnt, (core, st, rr, win), 1)
    cap = cnt.max(axis=0)
    slots = (cap + 127) // 128

    nb = int(slots.sum())
    TOK = nb * 128
    gidx = np.zeros((nc_, 128, TOK // 16), np.int16)
    dstv = np.full((nc_, 128, nb), -1.0, np.float32)
    wv = np.zeros((nc_, 128, nb), np.float32)

    cell_g0 = np.zeros((c.NST, c.NRES, c.NW), np.int64)
    g = 0
    for s in range(c.NST):
        for e in range(c.NRES):
            for b in range(c.NW):
                cell_g0[s, e, b] = g
                g += slots[s, e, b]

    ti = np.arange(TOK)
    rows16 = (ti % 16)[None, :] + 16 * np.arange(8)[:, None]
    cols16 = ti // 16

    key = ((core * c.NST + st) * c.NRES + rr) * c.NW + win
    order = np.lexsort((gix, key))
    ksort = key[order]
    nkey = nc_ * c.NST * c.NRES * c.NW
    bounds = np.searchsorted(ksort, np.arange(nkey + 1))
    for ci in range(nc_):
        g_lin = np.zeros(TOK, np.int16)
        d_lin = np.full(TOK, -1.0, np.float32)
        w_lin = np.zeros(TOK, np.float32)
        for s in range(c.NST):
            for e in range(c.NRES):
                for b in range(c.NW):
                    kk = ((ci * c.NST + s) * c.NRES + e) * c.NW + b
                    sel = order[bounds[kk]:bounds[kk + 1]]
                    n = len(sel)
                    t0 = int(cell_g0[s, e, b]) * 128
                    g_lin[t0:t0 + n] = gix[sel].astype(np.int16)
                    d_lin[t0:t0 + n] = qq[sel].astype(np.float32)
                    w_lin[t0:t0 + n] = w[sel]
                    ns_ = int(slots[s, e, b])
                    for q0_ in range(0, ns_, 8):
                        pos = t0 + q0_ * 128
                        if g_lin[pos] < 0:
                            g_lin[pos] = 0
        for gg in range(8):
            gidx[ci, rows16[gg], cols16] = g_lin
        dstv[ci, ti % 128, ti // 128] = d_lin
        wv[ci, ti % 128, ti // 128] = w_lin
    return gidx, dstv, wv, slots


# ----------------------------------------------------------------------
# device program
# ----------------------------------------------------------------------

def build_nc(cfg, slots):
    c = cfg
    nc = bacc.Bacc(None, target_bir_lowering=False, debug=False,
                   num_swdge_queues=NQUEUES)
    NT, NSP, C_, H = c.NT, c.NSP, c.C, c.H
    L = c.L
    NB = int(slots.sum())
    TOK = NB * 128

    def dram_in(name, shape, dt=F32):
        return nc.declare_dram_parameter(name, shape, dt, isOutput=False)

    xin_T = dram_in("xin_T", [H, NT, 128], FP16)
    d_T = dram_in("d_T", [128, NT, C_], F32)
    p_T = dram_in("p_T", [C_, NSP], FP16)
    gidx = dram_in("gidx", [128, TOK // 16], I16)
    dstv = dram_in("dstv", [128, NB], F32)
    wv = dram_in("wv", [128, NB], F32)
    lin0w = dram_in("lin0w", [H, H])
    lin0b = dram_in("lin0b", [H, 1])
    lin1w = dram_in("lin1w", [H, C_])
    lin1b = dram_in("lin1b", [C_, 1])
    wceff = dram_in("wceff", [L, H, H])
    cma45 = dram_in("cma45", [C_, C_])
    cmat45 = dram_in("cmat45", [C_, C_])
    i47 = dram_in("i47", [C_, C_])
    ident = dram_in("ident", [128, 128])
    iota = dram_in("iota", [128, c.SW], FP16)
    out_T = nc.declare_dram_parameter("out_T", [C_, NSP], F32, isOutput=True)

    # internal DRAM
    x_rep = [[nc.dram_tensor(f"x_rep{h}_{pbuf}", [c.REG, H], FP16,
                             addr_space="Shared")
              for pbuf in range(2)] for h in range(2)]
    sh = [[nc.dram_tensor(f"sh{i}{'AB'[h]}", [128, c.HT, H], FP16)
           for h in range(2)] for i in range(3)]
    # feature-major fp16 x buffers; xTh[0] holds x0 and is never overwritten
    xTh = [nc.dram_tensor(f"xTh{i}", [128, NT, H], FP16) for i in range(3)]
    cen_in = nc.dram_tensor("cen_in", [C_, H], F32)
    cen_out = nc.dram_tensor("cen_out", [C_, H], F32, addr_space="Shared")

    rg = [list(range(c.ncores))]
    CURS = [0, 1, 2, 1]
    NXTS = [1, 2, 1, 2]

    cell_g0 = np.zeros((c.NST, c.NRES, c.NW), np.int64)
    g = 0
    for s in range(c.NST):
        for e in range(c.NRES):
            for b in range(c.NW):
                cell_g0[s, e, b] = g
                g += slots[s, e, b]

    with tile.TileContext(nc) as tc:
        nc.gpsimd.load_library(library_config.mlp)
        with (
            tc.tile_pool(name="const", bufs=1) as cpool,
            tc.tile_pool(name="edge", bufs=1) as epool,
            tc.tile_pool(name="bslab", bufs=2) as bpool,
            tc.tile_pool(name="oslab", bufs=2) as opool,
            tc.tile_pool(name="gt", bufs=8) as gpool,
            tc.tile_pool(name="smat", bufs=6) as spool,
            tc.tile_pool(name="sb", bufs=4) as pool,
            tc.tile_pool(name="bank", bufs=c.CHST, space="PSUM") as bankp,
            tc.tile_pool(name="ps2", bufs=2, space="PSUM") as ps2p,
            tc.tile_pool(name="psn", bufs=1, space="PSUM") as psnp,
            tc.tile_pool(name="psA", bufs=1, space="PSUM") as psAp,
        ):
            # ---- resident constants ----
            lin0w_sb = cpool.tile([H, H], F32)
            nc.sync.dma_start(lin0w_sb[:], lin0w[:, :])
            lin0w_h = cpool.tile([H, H], FP16)
            nc.vector.tensor_copy(lin0w_h[:], lin0w_sb[:])
            lin0b_sb = cpool.tile([H, 1], F32)
            nc.sync.dma_start(lin0b_sb[:], lin0b[:, :])
            lin1w_sb = cpool.tile([H, C_], F32)
            nc.sync.dma_start(lin1w_sb[:], lin1w[:, :])
            lin1w_h = cpool.tile([H, C_], FP16)
            nc.vector.tensor_copy(lin1w_h[:], lin1w_sb[:])
            lin1b_sb = cpool.tile([C_, 1], F32)
            nc.sync.dma_start(lin1b_sb[:], lin1b[:, :])
            wceff_sb = cpool.tile([H, L * H], F32)
            for i in range(L):
                nc.sync.dma_start(wceff_sb[:, i * H:(i + 1) * H], wceff[i])
            cma_sb = cpool.tile([C_, C_], F32)
            nc.sync.dma_start(cma_sb[:], cma45[:, :])
            cmat_sb = cpool.tile([C_, C_], F32)
            nc.sync.dma_start(cmat_sb[:], cmat45[:, :])
            i47_sb = cpool.tile([C_, C_], F32)
            nc.sync.dma_start(i47_sb[:], i47[:, :])
            ident_sb = cpool.tile([128, 128], F32)
            nc.sync.dma_start(ident_sb[:], ident[:, :])
            identh_sb = cpool.tile([128, 128], FP16)
            nc.vector.tensor_copy(identh_sb[:], ident_sb[:])
            i45h_sb = cpool.tile([128, 128], FP16)
            nc.vector.tensor_scalar(i45h_sb[:], ident_sb[:], 0.45, None,
                                    mybir.AluOpType.mult)
            i10h_sb = cpool.tile([128, 128], FP16)
            nc.vector.tensor_scalar(i10h_sb[:], ident_sb[:], 0.1, None,
                                    mybir.AluOpType.mult)
            iota_sb = cpool.tile([128, c.SW], FP16)
            nc.sync.dma_start(iota_sb[:], iota[:, :])

            # ---- resident edge data ----
            gi_sb = epool.tile([128, TOK // 16], I16)
            nc.sync.dma_start(gi_sb[:], gidx[:, :])
            dv_sb = epool.tile([128, NB], F32)
            nc.sync.dma_start(dv_sb[:], dstv[:, :])
            wv_sb = epool.tile([128, NB], F32)
            nc.sync.dma_start(wv_sb[:], wv[:, :])
            d_res = epool.tile([128, NT, C_], F32)
            nc.sync.dma_start(d_res[:], d_T[:, :, :])
            cen0_sb = epool.tile([C_, H], F32)
            if ACT_MOD:
                # aux for ACT-engine S-build: t=Square((iota-d)/32),
                # S=Relu(w - 4096*w*t)
                dvn_sb = epool.tile([128, NB], F32)
                nc.vector.tensor_scalar(dvn_sb[:], dv_sb[:], -1.0 / 32, None,
                                        mybir.AluOpType.mult)
                wvn_sb = epool.tile([128, NB], F32)
                nc.vector.tensor_scalar(wvn_sb[:], wv_sb[:], -4096.0, None,
                                        mybir.AluOpType.mult)

            CHT = c.CHST * 4                       # tiles per chunk (16)

            def chunk_tiles(ch):
                t0 = ch * CHT
                return t0, min(CHT, NT - t0)

            def write_sh(dst_sh, shs, t0, ntl):
                if t0 + ntl <= c.HT:
                    nc.sync.dma_start(dst_sh[0][:, t0:t0 + ntl, :],
                                      shs[:, :ntl, :])
                elif t0 >= c.HT:
                    nc.sync.dma_start(
                        dst_sh[1][:, t0 - c.HT:t0 - c.HT + ntl, :],
                        shs[:, :ntl, :])
                else:
                    n1 = c.HT - t0
                    nc.sync.dma_start(dst_sh[0][:, t0:c.HT, :],
                                      shs[:, :n1, :])
                    nc.sync.dma_start(dst_sh[1][:, 0:ntl - n1, :],
                                      shs[:, n1:ntl, :])

            # ---- lin0 (also accumulates psA0 = d^T x0 in f32) ----
            psA = psAp.tile([C_, H], F32, tag="cen")
            for ch in range(c.NCH):
                t0, ntl = chunk_tiles(ch)
                xi = bpool.tile([128, CHT, H], FP16, tag="xTs")
                nc.sync.dma_start(xi[:, :ntl, :], xin_T[:, t0:t0 + ntl, :])
                xhs = opool.tile([128, CHT, H], FP16, tag="xhs")
                shs = opool.tile([128, CHT, H], FP16, tag="shs")
                for j in range(ntl):
                    t = t0 + j
                    ps0 = ps2p.tile([H, 128], F32, tag="b")
                    nc.tensor.matmul(ps0[:], lin0w_h[:], xi[:, j, :],
                                     start=True, stop=True)
                    nc.scalar.activation(xhs[:, j, :], ps0[:], AF.Relu,
                                         bias=lin0b_sb[:, 0:1])
                    xf = pool.tile([H, 128], F32, tag="xf")
                    nc.scalar.activation(xf[:], ps0[:], AF.Relu,
                                         bias=lin0b_sb[:, 0:1])
                    psn = psnp.tile([128, 128], F32, tag="bb")
                    nc.tensor.transpose(psn[:], xf[:], ident_sb[:])
                    nc.vector.tensor_copy(shs[:, j, :], psn[:])
                    sf = pool.tile([128, H], F32, tag="sf")
                    nc.vector.tensor_copy(sf[:], psn[:])
                    nc.tensor.matmul(psA[:], d_res[:, t, :], sf[:],
                                     start=(t == 0), stop=(t == NT - 1))
                nc.sync.dma_start(xTh[0][:, t0:t0 + ntl, :], xhs[:, :ntl, :])
                write_sh(sh[0], shs, t0, ntl)
            nc.vector.tensor_copy(cen0_sb[:], psA[:])

            # ---- layers ----
            qrr = [0]

            for li in range(L):
                cur, nxt = CURS[li], NXTS[li]
                pb = li % 2
                for h in range(2):
                    nc.gpsimd.collective_compute(
                        "AllGather", mybir.AluOpType.bypass,
                        replica_groups=rg,
                        ins=[sh[cur][h].ap().opt()],
                        outs=[x_rep[h][pb].ap().opt()],
                    )

                cenp = pool.tile([C_, H], F32, tag="cenp")
                if li == 0:
                    nc.vector.tensor_scalar(cenp[:], cen0_sb[:], 1.1, None,
                                            mybir.AluOpType.mult)
                else:
                    nc.vector.tensor_scalar(cenp[:], cen0_sb[:], 0.1, None,
                                            mybir.AluOpType.mult)
                    nc.vector.tensor_add(cenp[:], cenp[:], psA[:])
                nc.sync.dma_start(cen_in[:, :], cenp[:])
                nc.gpsimd.collective_compute(
                    "AllReduce", mybir.AluOpType.add, replica_groups=rg,
                    ins=[cen_in.ap().opt()], outs=[cen_out.ap().opt()],
                )
                cen = pool.tile([C_, H], F32, tag="cen_sb")
                nc.sync.dma_start(cen[:], cen_out[:, :])

                # - r_cls from centers (Gram trick), cma pre-scaled 0.45 -
                pst = ps2p.tile([128, C_], F32, tag="b")
                nc.tensor.transpose(pst[:, :], cen[:], ident_sb[:C_, :C_])
                cT = pool.tile([128, C_], F32, tag="cT")
                nc.vector.tensor_copy(cT[:], pst[:, :])
                psg = ps2p.tile([C_, C_], F32, tag="b")
                nc.tensor.matmul(psg[:], cT[:], cT[:], start=True, stop=True)
                gg = pool.tile([C_, C_], F32, tag="gg")
                nc.vector.tensor_copy(gg[:], psg[:])
                gd = pool.tile([C_, C_], F32, tag="gd")
                nc.vector.tensor_mul(gd[:], gg[:], i47_sb[:])
                n2 = pool.tile([C_, 1], F32, tag="n2")
                nc.vector.reduce_sum(n2[:], gd[:], AxisListType.X)
                t1 = pool.tile([C_, C_], F32, tag="t1")
                nc.vector.tensor_scalar(t1[:], gg[:], -1.0, n2[:, 0:1],
                                        mybir.AluOpType.mult,
                                        mybir.AluOpType.add)
                ps1 = ps2p.tile([C_, C_], F32, tag="b")
                nc.tensor.transpose(ps1[:], t1[:], ident_sb[:C_, :C_])
                nrm = pool.tile([C_, C_], F32, tag="nrm")
                nc.vector.tensor_add(nrm[:], t1[:], ps1[:])
                nc.vector.tensor_relu(nrm[:], nrm[:])
                nc.vector.tensor_add(nrm[:], nrm[:], i47_sb[:])
                rn = pool.tile([C_, C_], F32, tag="rn")
                nc.scalar.sqrt(rn[:], nrm[:])
                inv = pool.tile([C_, C_], F32, tag="inv")
                nc.vector.reciprocal(inv[:], rn[:])
                amat = pool.tile([C_, C_], F32, tag="amat")
                nc.vector.tensor_mul(amat[:], cma_sb[:], inv[:])
                atm = pool.tile([C_, C_], F32, tag="atm")
                nc.vector.tensor_mul(atm[:], cmat_sb[:], inv[:])
                rs = pool.tile([C_, 1], F32, tag="rs")
                nc.vector.reduce_sum(rs[:], amat[:], AxisListType.X)
                psm = ps2p.tile([C_, H], F32, tag="b")
                nc.tensor.matmul(psm[:], atm[:], cen[:], start=True, stop=True)
                rcls = pool.tile([C_, H], F32, tag="rcls")
                nc.vector.tensor_scalar(rcls[:], cen[:], rs[:, 0:1], None,
                                        mybir.AluOpType.mult)
                nc.vector.tensor_sub(rcls[:], rcls[:], psm[:])
                rclsh = pool.tile([C_, H], FP16, tag="rclsh")
                nc.vector.tensor_copy(rclsh[:], rcls[:])

                # - propagate + pass B, chunked -
                for ch in range(c.NCH):
                    t0, ntl = chunk_tiles(ch)
                    st0 = ch * c.CHST
                    nst = min(c.CHST, c.NST - st0)
                    xTs = bpool.tile([128, CHT, H], FP16, tag="xTs")
                    nc.sync.dma_start(xTs[:, :ntl, :],
                                      xTh[cur][:, t0:t0 + ntl, :])
                    x0s2 = bpool.tile([128, CHT, H], FP16, tag="x0s2")
                    nc.sync.dma_start(x0s2[:, :ntl, :],
                                      xTh[0][:, t0:t0 + ntl, :])
                    pts = bpool.tile([C_, CHT * 128], FP16, tag="pts")
                    nc.sync.dma_start(pts[:, :ntl * 128],
                                      p_T[:, t0 * 128:(t0 + ntl) * 128])

                    banks = []
                    for si in range(nst):
                        st = st0 + si
                        stw_st = min(c.STW, NSP - st * c.STW)
                        sw_st = stw_st // c.NRES
                        bank = bankp.tile([H, c.STW], F32, tag="bank")
                        banks.append(bank)
                        first = [True]
                        jlast = min(3, NT - 1 - st * 4)
                        for e, wnd in [(e_, w_) for w01 in (0, 1)
                                       for e_ in range(c.NRES)
                                       for w_ in (2 * w01, 2 * w01 + 1)]:
                            bsl = bank[:, e * sw_st:(e + 1) * sw_st]
                            if True:
                                g0 = int(cell_g0[st, e, wnd])
                                ng = int(slots[st, e, wnd])
                                if ng == 0:
                                    continue
                                rep = x_rep[wnd // 2][pb]
                                base = (wnd % 2) * c.W
                                gts = []
                                for q0 in range(0, ng, 8):
                                    q1 = min(q0 + 8, ng)
                                    ntok = (q1 - q0) * 128
                                    gt = gpool.tile([128, 8, H], FP16,
                                                    tag="g")
                                    nc.gpsimd.dma_gather(
                                        gt[:, :q1 - q0, :],
                                        rep[base:base + c.W, :],
                                        gi_sb[:, (g0 + q0) * 8:(g0 + q1) * 8],
                                        num_idxs=ntok, num_idxs_reg=ntok,
                                        elem_size=H,
                                        queue_num=qrr[0] % NQUEUES,
                                    )
                                    qrr[0] += 1
                                    gts.append((gt, q0, q1 - q0))
                                for (gt, q0, nq) in gts:
                                    for k in range(nq):
                                        gl = g0 + q0 + k
                                        S = spool.tile([128, c.SW], FP16,
                                                       tag="S")
                                        if ACT_MOD and gl % 5 < 3:
                                            St = spool.tile([128, c.SW],
                                                            FP16, tag="St")
                                            nc.scalar.activation(
                                                St[:], iota_sb[:], AF.Square,
                                                bias=dvn_sb[:, gl:gl + 1],
                                                scale=1.0 / 32)
                                            nc.scalar.activation(
                                                S[:], St[:], AF.Relu,
                                                bias=wv_sb[:, gl:gl + 1],
                                                scale=wvn_sb[:, gl:gl + 1])
                                        else:
                                            nc.vector.tensor_scalar(
                                                S[:], iota_sb[:],
                                                dv_sb[:, gl:gl + 1],
                                                wv_sb[:, gl:gl + 1],
                                                mybir.AluOpType.is_equal,
                                                mybir.AluOpType.mult)
                                        nc.tensor.matmul(
                                            bsl, gt[:, k, :], S[:, :sw_st],
                                            start=first[0], stop=False,
                                            skip_group_check=True)
                                        first[0] = False
                        for j in range(4):
                            t = st * 4 + j
                            if t >= NT:
                                break
                            jj = t - t0
                            sl = bank[:, j * 128:(j + 1) * 128]
                            nc.tensor.matmul(sl, rclsh[:],
                                             pts[:, jj * 128:(jj + 1) * 128],
                                             start=first[0], stop=False,
                                             skip_group_check=True)
                            nc.tensor.matmul(sl, i45h_sb[:], xTs[:, jj, :],
                                             start=False, stop=False,
                                             skip_group_check=True)
                            nc.tensor.matmul(sl, i10h_sb[:], x0s2[:, jj, :],
                                             start=False, stop=(j == jlast),
                                             skip_group_check=True)
                            first[0] = False

                    # pass B compute per tile (also accumulates next
                    # layer's centers from f32 data); the last layer
                    # feeds lin1 directly instead of storing x
                    if ch == 0 and li < L - 1:
                        psA = psAp.tile([C_, H], F32, tag="cen")
                    if li < L - 1:
                        xhs = opool.tile([128, CHT, H], FP16, tag="xhs")
                        shs = opool.tile([128, CHT, H], FP16, tag="shs")
                    else:
                        ots = opool.tile([C_, CHT * 128], F32, tag="ots")
                    for si in range(nst):
                        st = st0 + si
                        for j in range(4):
                            t = st * 4 + j
                            if t >= NT:
                                break
                            jj = t - t0
                            u = pool.tile([H, 128], F32, tag="u")
                            nc.vector.tensor_copy(
                                u[:], banks[si][:, j * 128:(j + 1) * 128])
                            ps2 = ps2p.tile([H, 128], F32, tag="b")
                            nc.tensor.matmul(ps2[:],
                                             wceff_sb[:, li * H:(li + 1) * H],
                                             u[:], start=True, stop=True)
                            if li < L - 1:
                                nc.scalar.activation(xhs[:, jj, :], ps2[:],
                                                     AF.Relu)
                                xf = pool.tile([H, 128], F32, tag="xf")
                                nc.scalar.activation(xf[:], ps2[:], AF.Relu)
                                psn = psnp.tile([128, 128], F32, tag="bb")
                                nc.tensor.transpose(psn[:], xf[:],
                                                    ident_sb[:])
                                nc.vector.tensor_copy(shs[:, jj, :], psn[:])
                                sf = pool.tile([128, H], F32, tag="sf")
                                nc.vector.tensor_copy(sf[:], psn[:])
                                nc.tensor.matmul(psA[:], d_res[:, t, :],
                                                 sf[:], start=(t == 0),
                                                 stop=(t == NT - 1))
                            else:
                                xh = pool.tile([H, 128], FP16, tag="xh")
                                nc.scalar.activation(xh[:], ps2[:], AF.Relu)
                                psf = ps2p.tile([C_, 128], F32, tag="b")
                                nc.tensor.matmul(psf[:], lin1w_h[:], xh[:],
                                                 start=True, stop=True)
                                nc.vector.tensor_scalar(
                                    ots[:, jj * 128:(jj + 1) * 128],
                                    psf[:], lin1b_sb[:, 0:1], None,
                                    mybir.AluOpType.add)
                    if li < L - 1:
                        nc.sync.dma_start(xTh[nxt][:, t0:t0 + ntl, :],
                                          xhs[:, :ntl, :])
                        write_sh(sh[nxt], shs, t0, ntl)
                    else:
                        nc.sync.dma_start(
                            out_T[:, t0 * 128:(t0 + ntl) * 128],
                            ots[:, :ntl * 128])

    nc.compile()
    return nc


def _load_sh_slab(nc, c, shp, dest, t0, ntl):
    """Load node-major tiles [t0, t0+ntl) from half-split sh into dest."""
    if t0 + ntl <= c.HT:
        nc.sync.dma_start(dest[:, :ntl, :], shp[0][:, t0:t0 + ntl, :])
    elif t0 >= c.HT:
        nc.sync.dma_start(dest[:, :ntl, :],
                          shp[1][:, t0 - c.HT:t0 - c.HT + ntl, :])
    else:
        n1 = c.HT - t0
        nc.sync.dma_start(dest[:, :n1, :], shp[0][:, t0:c.HT, :])
        nc.sync.dma_start(dest[:, n1:ntl, :], shp[1][:, 0:ntl - n1, :])


# ----------------------------------------------------------------------
# host wrapper
# ----------------------------------------------------------------------

def _prep_inputs(cfg, inputs):
    c = cfg
    x = np.asarray(inputs["x"], np.float32)
    label = np.asarray(inputs["label"], np.int64)
    p = np.asarray(inputs["p"], np.float32)
    cm = np.asarray(inputs["cm"], np.float32)
    lin0_w = np.asarray(inputs["lin0_w"], np.float32)
    lin0_b = np.asarray(inputs["lin0_b"], np.float32)
    lin1_w = np.asarray(inputs["lin1_w"], np.float32)
    lin1_b = np.asarray(inputs["lin1_b"], np.float32)
    conv_w = np.asarray(inputs["conv_w"], np.float32)

    gidx, dstv, wv, slots = _prep_edges(cfg, inputs["edge_index"],
                                        inputs["edge_weight"])

    cnt = np.bincount(label, minlength=c.C).astype(np.float32)
    cnt = np.maximum(cnt, 1.0)
    cma = cm[:, 0, :] * (c.rsl * (1.0 - c.alpha))      # 0.45 fold
    i47 = np.eye(c.C, dtype=np.float32)
    ident = np.eye(128, dtype=np.float32)
    iota = np.tile(np.arange(c.SW, dtype=np.float16)[None, :], (128, 1))
    wceff = np.zeros((c.L, c.H, c.H), np.float32)
    for i in range(c.L):
        beta = float(np.log(c.theta / (i + 1) + 1.0))
        wceff[i] = (1.0 - beta) * np.eye(c.H, dtype=np.float32) \
            + beta * conv_w[i]

    perm = _node_perm(c)
    pidx = perm[np.arange(c.NS)]
    in_maps = []
    for ci in range(c.ncores):
        r0 = ci * c.NS
        xs = np.zeros((c.NSP, c.H), np.float32)
        xs[pidx] = x[r0:r0 + c.NS]
        lab = label[r0:r0 + c.NS]
        d_t = np.zeros((c.NSP, c.C), np.float32)
        d_t[pidx, lab] = 1.0 / cnt[lab]
        p_pad = np.zeros((c.NSP, c.C), np.float32)
        p_pad[pidx] = p[r0:r0 + c.NS]
        in_maps.append({
            "xin_T": np.ascontiguousarray(xs.T).reshape(
                c.H, c.NT, 128).astype(np.float16),
            "d_T": np.ascontiguousarray(
                d_t.reshape(c.NT, 128, c.C).transpose(1, 0, 2)),
            "p_T": np.ascontiguousarray(p_pad.T).astype(np.float16),
            "gidx": gidx[ci], "dstv": dstv[ci], "wv": wv[ci],
            "lin0w": lin0_w, "lin0b": lin0_b.reshape(-1, 1),
            "lin1w": lin1_w, "lin1b": lin1_b.reshape(-1, 1),
            "wceff": wceff, "cma45": cma,
            "cmat45": np.ascontiguousarray(cma.T),
            "i47": i47, "ident": ident, "iota": iota,
        })
    return in_maps, slots


_BUILT = {}


def kernel(**inputs):
    cfg = DEF
    in_maps, slots = _prep_inputs(cfg, inputs)
    key = "default"
    if key not in _BUILT:
        _BUILT[key] = build_nc(cfg, slots)
    nc = _BUILT[key]
    res = bass_utils.run_bass_kernel_spmd(nc, in_maps,
                                          core_ids=list(range(cfg.ncores)))
    pidx = _node_perm(cfg)[np.arange(cfg.NS)]
    outs = [res.results[ci]["out_T"].T[pidx] for ci in range(cfg.ncores)]
    return np.ascontiguousarray(np.concatenate(outs, 0).astype(np.float32))
